# revision 1
# baseline (speedup 1.0000x reference)
"""Distributed Trainium2 Bass kernel for nn_Attention_74732430950409.

Single-query MHA with RoPE'd keys/values, 4 projection weights folded
algebraically onto the tiny query side:

  qtil[h,:] = (((x @ Wq.T) @ Wq_mha.T)[h] @ Wk_mha[h]) @ Wk        (16, 2048)
  logits[s,h] = rope(keys)[s,:] . qtil[h,:] / sqrt(128)
  w = exp(logits)          (no max subtraction; |logits| < ~6)
  u[h,:] = sum_s w[s,h] * rope(states)[s,:]                        (16, 2048)
  l[h]   = sum_s w[s,h]
  z[h,:]  = (u[h,:] @ Wv.T) / l[h]                                 (16, 2048)
  attn[h,:] = z[h,:] @ Wv_mha[h].T                                 (16, 128)
  out = attn.flat @ Wo.T + x

Sequence-sharded across 8 cores (1024 rows each); all heavy tensors are
streamed once from HBM. Five collectives: AG(q), AG(tmp), AR(qtilT),
AR(u|l), AR(attnT). Compute dtype bf16 (f32 PSUM accum).
"""

import sys
import numpy as np

for p in ("/opt/trn_rl_repo",):
    if p not in sys.path:
        sys.path.insert(0, p)

import ml_dtypes

BF16 = ml_dtypes.bfloat16

NUM_HEADS = 16
QK = 2048
VO = 2048
S = 8192
NC = 8
S_LOC = S // NC          # 1024
SH = VO // NC            # 256 rows per core of each weight
DQ = QK // NUM_HEADS     # 128
HALF = VO // 2           # 1024
ROPE_THETA = 10000.0

_cache = {}


def _build():
    import concourse.bass as bass
    import concourse.mybir as mybir
    import concourse.bacc as bacc
    import concourse.tile as tile

    f32 = mybir.dt.float32
    bf16 = mybir.dt.bfloat16
    AF = mybir.ActivationFunctionType
    ALU = mybir.AluOpType
    PSUM = bass.MemorySpace.PSUM

    nc = bacc.Bacc(None, target_bir_lowering=False)

    # ---------------- DRAM parameters (per-core shards) ----------------
    keysT_d = nc.dram_tensor("keysT", [QK, S_LOC], bf16, kind="ExternalInput")
    states_d = nc.dram_tensor("states", [S_LOC, VO], bf16, kind="ExternalInput")
    xq_d = nc.dram_tensor("xq", [QK], bf16, kind="ExternalInput")
    identb_d = nc.dram_tensor("identb", [128, 128], bf16, kind="ExternalInput")
    xo_d = nc.dram_tensor("xo", [SH], f32, kind="ExternalInput")
    ident_d = nc.dram_tensor("ident", [128, 128], f32, kind="ExternalInput")
    ck_d = nc.dram_tensor("ck", [HALF, S_LOC], bf16, kind="ExternalInput")
    sk_d = nc.dram_tensor("sk", [HALF, S_LOC], bf16, kind="ExternalInput")
    cs_d = nc.dram_tensor("cs", [S_LOC, HALF], bf16, kind="ExternalInput")
    ss_d = nc.dram_tensor("ss", [S_LOC, HALF], bf16, kind="ExternalInput")
    wqT_d = nc.dram_tensor("wqT", [QK, SH], bf16, kind="ExternalInput")
    wqmC_d = nc.dram_tensor("wqmC", [SH, QK], bf16, kind="ExternalInput")
    wkmC_d = nc.dram_tensor("wkmC", [QK, SH], bf16, kind="ExternalInput")
    wk_d = nc.dram_tensor("wk", [SH, VO], bf16, kind="ExternalInput")
    wvT_d = nc.dram_tensor("wvT", [VO, SH], bf16, kind="ExternalInput")
    wvm_d = nc.dram_tensor("wvm", [SH, VO], bf16, kind="ExternalInput")
    woT_d = nc.dram_tensor("woT", [VO, SH], bf16, kind="ExternalInput")
    out_d = nc.dram_tensor("out", [1, SH], f32, kind="ExternalOutput")
    DEBUG = _cache.get("debug", False)
    if DEBUG:
        dqt_d = nc.dram_tensor("dbg_qt", [VO, NUM_HEADS], f32, kind="ExternalOutput")
        dw_d = nc.dram_tensor("dbg_w", [NUM_HEADS, S_LOC], f32, kind="ExternalOutput")
        du_d = nc.dram_tensor("dbg_u", [128, 16 * NUM_HEADS + 1], f32, kind="ExternalOutput")
        dat_d = nc.dram_tensor("dbg_at", [DQ, NUM_HEADS], f32, kind="ExternalOutput")
        dqh_d = nc.dram_tensor("dbg_qh", [1, QK], f32, kind="ExternalOutput")
        dtT_d = nc.dram_tensor("dbg_tT", [128, 2, NUM_HEADS], f32, kind="ExternalOutput")

    RG = [list(range(NC))]
    SCALE = 1.0 / float(np.sqrt(DQ))

    with tile.TileContext(nc) as tc:
        with (
            tc.tile_pool(name="kbuf", bufs=16) as kbuf,
            tc.tile_pool(name="sbuf_s", bufs=8) as sbuf_s,
            tc.tile_pool(name="tabs", bufs=1) as tabs,
            tc.tile_pool(name="wts", bufs=4) as wts,
            tc.tile_pool(name="tmps", bufs=8) as tmps,
            tc.tile_pool(name="small", bufs=1) as small,
            tc.tile_pool(name="psA", bufs=5, space=PSUM) as psA,
            tc.tile_pool(name="psB", bufs=3, space=PSUM) as psB,
            tc.tile_pool(name="dram", bufs=1, space="DRAM") as dram,
        ):
            # ---------------- collective bounce buffers ----------------
            bqh_in = dram.tile([128, NUM_HEADS], f32)
            bqh_out = dram.tile([128, NUM_HEADS], f32)
            bqt_in = dram.tile([128, 16 * NUM_HEADS], bf16)
            bqt_out = dram.tile([128, 16 * NUM_HEADS], bf16)
            bu_in = dram.tile([128, 16 * NUM_HEADS + 1], f32)
            bu_out = dram.tile([128, 16 * NUM_HEADS + 1], f32)
            bat_in = dram.tile([DQ, NUM_HEADS], f32)
            bqh = dram.tile([1, SH], f32)
            bat_out = dram.tile([DQ, NUM_HEADS], f32)


            # ---------------- small persistent SBUF tiles ----------------
            x_sb = small.tile([128, 16], bf16, tag="x")          # x as [k%128, kc]
            ident_f = small.tile([128, 128], f32, tag="idf")
            ident_b = small.tile([128, 128], bf16, tag="idb")
            qT_sb = small.tile([128, 2], bf16, tag="qT")         # local q shard, transposed
            qhT_sb = small.tile([128, 16], bf16, tag="qhT")      # full qh, transposed
            tmpT_sb = small.tile([128, 2, NUM_HEADS], bf16, tag="tmpT")
            qtp_sb = small.tile([128, 16, NUM_HEADS], bf16, tag="qtp")
            qtilT_sb = small.tile([128, 16, NUM_HEADS], bf16, tag="qtilT")
            w_sb = small.tile([NUM_HEADS, S_LOC], bf16, tag="w")
            l0_sb = small.tile([NUM_HEADS, 1], f32, tag="l0")
            l1_sb = small.tile([NUM_HEADS, 1], f32, tag="l1")
            lp_sb = small.tile([NUM_HEADS, 1], f32, tag="lp")
            wT_sb = small.tile([128, 8, NUM_HEADS], bf16, tag="wT")
            u_sb = small.tile([NUM_HEADS, VO], f32, tag="u")
            uT_sb = small.tile([128, 16, NUM_HEADS], f32, tag="uT")
            uT_bf = small.tile([128, 16, NUM_HEADS], bf16, tag="uTb")
            l_sb = small.tile([NUM_HEADS, 1], f32, tag="l")
            rl_sb = small.tile([NUM_HEADS, 1], f32, tag="rl")
            z_sb = small.tile([NUM_HEADS, SH], bf16, tag="z")
            zT_sb = small.tile([128, 2, NUM_HEADS], bf16, tag="zT")
            atT_sb = small.tile([128, NUM_HEADS], f32, tag="atT")
            atT_bf = small.tile([128, NUM_HEADS], bf16, tag="atTb")
            xo_sb = small.tile([1, SH], f32, tag="xo")
            out_sb = small.tile([1, SH], f32, tag="out")

            # ---------------- q-path weights ----------------
            wqT_sb = wts.tile([128, 16, SH], bf16, tag="w8k")
            wqmC_sb = wts.tile([128, 2, QK], bf16, tag="w8k")
            wkmC_sb = wts.tile([128, 16, SH], bf16, tag="w8k")
            wk_sb = wts.tile([128, 2, VO], bf16, tag="w8k")
            nc.sync.dma_start(wqT_sb[:], wqT_d[:, :].rearrange("(kc p) n -> p kc n", p=128))
            nc.sync.dma_start(wqmC_sb[:], wqmC_d[:, :].rearrange("(nc2 p) m -> p nc2 m", p=128))
            nc.sync.dma_start(wkmC_sb[:], wkmC_d[:, :].rearrange("(h p) j -> p h j", p=128))
            nc.sync.dma_start(wk_sb[:], wk_d[:, :].rearrange("(jc p) i -> p jc i", p=128))


            # x / identity
            nc.sync.dma_start(x_sb[:], xq_d[:].rearrange("(f p) -> p f", p=128))
            nc.sync.dma_start(ident_f[:], ident_d[:, :])
            nc.sync.dma_start(ident_b[:], identb_d[:, :])
            nc.sync.dma_start(xo_sb[:], xo_d[:].rearrange("(a n) -> a n", a=1))

            # ---------------- qT = (x @ Wq.T)^T  (local shard, [128, 2]) ----------------
            for nc2 in range(2):
                qt_ps2 = psB.tile([128, 1], f32, tag="pB", name=f"qt_ps2_{nc2}")
                for kc in range(16):
                    nc.tensor.matmul(qt_ps2[:], wqT_sb[:, kc, nc2 * 128 : (nc2 + 1) * 128],
                                     x_sb[:, kc : kc + 1], start=(kc == 0), stop=(kc == 15))
                nc.scalar.activation(qT_sb[:, nc2 : nc2 + 1], qt_ps2[:], AF.Copy)

            # ---------------- qhT partial [d, h] = (q_shard @ Wq_mha[:, shard].T)^T ----
            qhT_ps = psB.tile([128, NUM_HEADS], f32, tag="pB")
            for h in range(NUM_HEADS):
                for nc2 in range(2):
                    nc.tensor.matmul(qhT_ps[:, h : h + 1],
                                     wqmC_sb[:, nc2, h * 128 : (h + 1) * 128],
                                     qT_sb[:, nc2 : nc2 + 1],
                                     start=(nc2 == 0), stop=(nc2 == 1))
            qhTp_sb = small.tile([128, NUM_HEADS], f32, tag="qhTp")
            nc.scalar.activation(qhTp_sb[:], qhT_ps[:], AF.Copy)
            nc.sync.dma_start(bqh_in[:], qhTp_sb[:])
            nc.gpsimd.collective_compute(
                "AllReduce", ALU.add, ins=[bqh_in[:].opt()], outs=[bqh_out[:].opt()],
                replica_groups=RG)
            nc.gpsimd.dma_start(qhT_sb[:], bqh_out[:, :])

            # ---------------- tmpT[j, h] local j-shard ----------------
            tmpT_ps = [psB.tile([128, NUM_HEADS], f32, tag="pB", name=f"tmpT_ps{j}")
                       for j in range(2)]
            for h in range(NUM_HEADS):
                for jc in range(2):
                    nc.tensor.matmul(tmpT_ps[jc][:, h : h + 1],
                                     wkmC_sb[:, h, jc * 128 : (jc + 1) * 128],
                                     qhT_sb[:, h : h + 1], start=True, stop=True)
            for jc in range(2):
                nc.scalar.activation(tmpT_sb[:, jc, :], tmpT_ps[jc][:], AF.Copy)

            # ---------------- qtilT partial = Wk_shard.T-contract ----------------
            for ic in range(16):
                qt_ps = psB.tile([128, NUM_HEADS], f32, tag="pB")
                for jc in range(2):
                    nc.tensor.matmul(qt_ps[:], wk_sb[:, jc, ic * 128 : (ic + 1) * 128],
                                     tmpT_sb[:, jc, :], start=(jc == 0), stop=(jc == 1))
                nc.scalar.activation(qtp_sb[:, ic, :], qt_ps[:], AF.Copy)
            nc.sync.dma_start(bqt_in[:, :].rearrange("p (ic h) -> p ic h", ic=16), qtp_sb[:])
            nc.gpsimd.collective_compute(
                "AllReduce", ALU.add, ins=[bqt_in[:].opt()], outs=[bqt_out[:].opt()],
                replica_groups=RG)
            nc.gpsimd.dma_start(
                qtilT_sb[:], bqt_out[:, :].rearrange("p (ic h) -> p ic h", ic=16))

            # ---------------- stream keys (transposed layout) + tables ----------------
            ck_sb = tabs.tile([128, 8, S_LOC], bf16, tag="ck")
            sk_sb = tabs.tile([128, 8, S_LOC], bf16, tag="sk")
            nc.sync.dma_start(ck_sb[:], ck_d[:, :].rearrange("(t p) s -> p t s", p=128))
            nc.sync.dma_start(sk_sb[:], sk_d[:, :].rearrange("(t p) s -> p t s", p=128))

            kt = []
            for ci in range(16):
                t = kbuf.tile([128, S_LOC], bf16, tag="kt")
                nc.sync.dma_start(t[:], keysT_d[ci * 128 : (ci + 1) * 128, :])
                kt.append(t)

            # rope keys in place (pairs ci, ci+8)
            for ci in range(8):
                a, b = kt[ci], kt[ci + 8]
                t1 = tmps.tile([128, S_LOC], bf16, tag="rt")
                t2 = tmps.tile([128, S_LOC], bf16, tag="rt")
                t3 = tmps.tile([128, S_LOC], bf16, tag="rt")
                t4 = tmps.tile([128, S_LOC], bf16, tag="rt")
                nc.vector.tensor_mul(t1[:], a[:], ck_sb[:, ci, :])
                nc.vector.tensor_mul(t2[:], b[:], sk_sb[:, ci, :])
                nc.vector.tensor_mul(t3[:], b[:], ck_sb[:, ci, :])
                nc.vector.tensor_mul(t4[:], a[:], sk_sb[:, ci, :])
                nc.vector.tensor_sub(a[:], t1[:], t2[:])
                nc.vector.tensor_add(b[:], t3[:], t4[:])

            # ---------------- logits + exp ----------------
            for sc in range(2):
                lg_ps = psA.tile([NUM_HEADS, 512], f32, tag="pA")
                for ic in range(16):
                    nc.tensor.matmul(lg_ps[:], qtilT_sb[:, ic, :],
                                     kt[ic][:, sc * 512 : (sc + 1) * 512],
                                     start=(ic == 0), stop=(ic == 15))
                nc.scalar.activation(w_sb[:, sc * 512 : (sc + 1) * 512], lg_ps[:],
                                     AF.Exp, scale=SCALE,
                                     accum_out=(l0_sb[:] if sc == 0 else l1_sb[:]))
            nc.vector.tensor_add(lp_sb[:], l0_sb[:], l1_sb[:])

            # wT via PE transpose: [16,128] slices -> [128,16]
            for sb in range(8):
                tr_ps = psB.tile([128, NUM_HEADS], bf16, tag="pB")
                nc.tensor.transpose(tr_ps[:], w_sb[:, sb * 128 : (sb + 1) * 128],
                                    ident_b[0:NUM_HEADS, 0:NUM_HEADS])
                nc.scalar.activation(wT_sb[:, sb, :], tr_ps[:], AF.Copy)

            # ---------------- stream states + tables + rope ----------------
            cs_sb = tabs.tile([128, 8, HALF], bf16, tag="cs")
            ss_sb = tabs.tile([128, 8, HALF], bf16, tag="ss")
            nc.sync.dma_start(cs_sb[:], cs_d[:, :].rearrange("(t p) j -> p t j", p=128))
            nc.sync.dma_start(ss_sb[:], ss_d[:, :].rearrange("(t p) j -> p t j", p=128))

            st = []
            for sb in range(8):
                t = sbuf_s.tile([128, VO], bf16, tag="st")
                nc.sync.dma_start(t[:], states_d[sb * 128 : (sb + 1) * 128, :])
                st.append(t)

            for sb in range(8):
                t = st[sb]
                t1 = tmps.tile([128, HALF], bf16, tag="rt")
                t2 = tmps.tile([128, HALF], bf16, tag="rt")
                t3 = tmps.tile([128, HALF], bf16, tag="rt")
                t4 = tmps.tile([128, HALF], bf16, tag="rt")
                nc.vector.tensor_mul(t1[:], t[:, 0:HALF], cs_sb[:, sb, :])
                nc.vector.tensor_mul(t2[:], t[:, HALF:VO], ss_sb[:, sb, :])
                nc.vector.tensor_mul(t3[:], t[:, HALF:VO], cs_sb[:, sb, :])
                nc.vector.tensor_mul(t4[:], t[:, 0:HALF], ss_sb[:, sb, :])
                nc.vector.tensor_sub(t[:, 0:HALF], t1[:], t2[:])
                nc.vector.tensor_add(t[:, HALF:VO], t3[:], t4[:])

            # ---------------- u = wT.T @ states_pe ----------------
            u_ps = [psA.tile([NUM_HEADS, 512], f32, tag="pA", name=f"u_ps{i}")
                    for i in range(4)]
            for sb in range(8):
                for nch in range(4):
                    nc.tensor.matmul(u_ps[nch][:], wT_sb[:, sb, :],
                                     st[sb][:, nch * 512 : (nch + 1) * 512],
                                     start=(sb == 0), stop=(sb == 7))
            for nch in range(4):
                nc.scalar.activation(u_sb[:, nch * 512 : (nch + 1) * 512],
                                     u_ps[nch][:], AF.Copy)

            # uT via PE transpose (f32)
            for ic in range(16):
                tr_ps = psB.tile([128, NUM_HEADS], f32, tag="pB")
                nc.tensor.transpose(tr_ps[:], u_sb[:, ic * 128 : (ic + 1) * 128],
                                    ident_f[0:NUM_HEADS, 0:NUM_HEADS])
                nc.scalar.activation(uT_sb[:, ic, :], tr_ps[:], AF.Copy)
            nc.sync.dma_start(bu_in[:, 0:256].rearrange("p (ic h) -> p ic h", ic=16), uT_sb[:])
            nc.sync.dma_start(bu_in[0:NUM_HEADS, 256:257], lp_sb[:])
            nc.gpsimd.collective_compute(
                "AllReduce", ALU.add, ins=[bu_in[:].opt()], outs=[bu_out[:].opt()],
                replica_groups=RG)
            nc.gpsimd.dma_start(
                uT_bf[:], bu_out[:, 0:256].rearrange("p (ic h) -> p ic h", ic=16))
            nc.sync.dma_start(l_sb[:], bu_out[0:NUM_HEADS, 256:257])
            nc.vector.reciprocal(rl_sb[:], l_sb[:])

            # ---------------- epilogue weights ----------------
            wvT_sb = wts.tile([128, 16, SH], bf16, tag="w8k")
            wvm_sb = wts.tile([128, 2, VO], bf16, tag="w8k")
            woT_sb = wts.tile([128, 16, SH], bf16, tag="w8k")
            nc.sync.dma_start(wvT_sb[:], wvT_d[:, :].rearrange("(ic p) j -> p ic j", p=128))
            nc.sync.dma_start(wvm_sb[:], wvm_d[:, :].rearrange("(jc p) m -> p jc m", p=128))
            nc.sync.dma_start(woT_sb[:], woT_d[:, :].rearrange("(mc p) n -> p mc n", p=128))

            # ---------------- z = (u @ Wv.T) / l ----------------
            z_ps = psB.tile([NUM_HEADS, SH], f32, tag="pB")
            for ic in range(16):
                nc.tensor.matmul(z_ps[:], uT_bf[:, ic, :], wvT_sb[:, ic, :],
                                 start=(ic == 0), stop=(ic == 15))
            nc.scalar.activation(z_sb[:], z_ps[:], AF.Copy, scale=rl_sb[:])

            # zT
            for jc in range(2):
                tr_ps = psB.tile([128, NUM_HEADS], bf16, tag="pB")
                nc.tensor.transpose(tr_ps[:], z_sb[:, jc * 128 : (jc + 1) * 128],
                                    ident_b[0:NUM_HEADS, 0:NUM_HEADS])
                nc.scalar.activation(zT_sb[:, jc, :], tr_ps[:], AF.Copy)

            # ---------------- attn partial ----------------
            at_ps = psB.tile([128, NUM_HEADS], f32, tag="pB")
            for h in range(NUM_HEADS):
                for jc in range(2):
                    nc.tensor.matmul(at_ps[:, h : h + 1],
                                     wvm_sb[:, jc, h * 128 : (h + 1) * 128],
                                     zT_sb[:, jc, h : h + 1],
                                     start=(jc == 0), stop=(jc == 1))
            nc.scalar.activation(atT_sb[:], at_ps[:], AF.Copy)
            nc.sync.dma_start(bat_in[:], atT_sb[:])
            nc.gpsimd.collective_compute(
                "AllReduce", ALU.add, ins=[bat_in[:].opt()], outs=[bat_out[:].opt()],
                replica_groups=RG)
            nc.gpsimd.dma_start(atT_bf[:], bat_out[:, :])

            if DEBUG:
                nc.sync.dma_start(dqh_d[:, :], bqh_out[:, :])
                nc.gpsimd.dma_start(dtT_d[:, :, :], tmpT_sb[:])
                nc.sync.dma_start(dqt_d[:, :], bqt_out[:, :])
                nc.gpsimd.dma_start(dw_d[:, :], w_sb[:])
                nc.sync.dma_start(du_d[:, :], bu_out[:, :])
                nc.sync.dma_start(dat_d[:, :], bat_out[:, :])

            # ---------------- out = attn @ Wo.T + x ----------------
            o_ps = psB.tile([1, SH], f32, tag="pB")
            for h in range(NUM_HEADS):
                nc.tensor.matmul(o_ps[:], atT_bf[:, h : h + 1], woT_sb[:, h, :],
                                 start=(h == 0), stop=(h == NUM_HEADS - 1))
            nc.vector.tensor_add(out_sb[:], o_ps[:], xo_sb[:])
            nc.sync.dma_start(out_d[:, :], out_sb[:])

    nc.compile()
    return nc


def _tables():
    # mimic reference: f32 angles, f32 cos/sin, then bf16
    half = HALF
    freqs = 1.0 / (ROPE_THETA ** (np.arange(half, dtype=np.float32) * 2.0 / VO))
    ang = np.outer(np.arange(S, dtype=np.float32), freqs).astype(np.float32)  # (S, half)
    return np.cos(ang), np.sin(ang)


def kernel(x, keys, states, Wq, Wk, Wv, Wq_mha, Wk_mha, Wv_mha, Wo):
    from concourse import bass_utils

    if "nc" not in _cache:
        _cache["nc"] = _build()
    nc = _cache["nc"]

    x = np.asarray(x, np.float32)
    keys = np.asarray(keys, np.float32)
    states = np.asarray(states, np.float32)
    cos_t, sin_t = _tables()

    ident = np.eye(128, dtype=np.float32)
    in_maps = []
    for c in range(NC):
        rs = slice(c * SH, (c + 1) * SH)
        ss_ = slice(c * S_LOC, (c + 1) * S_LOC)
        cosc = cos_t[ss_]            # (1024, 1024) [s_loc, j]
        sinc = sin_t[ss_]
        m = {
            "keysT": np.ascontiguousarray(keys[ss_].T).astype(BF16),
            "states": np.ascontiguousarray(states[ss_]).astype(BF16),
            "xq": x.astype(BF16),
            "identb": ident.astype(BF16),
            "xo": np.ascontiguousarray(x[rs]),
            "ident": ident,
            "ck": np.ascontiguousarray(cosc.T).astype(BF16),
            "sk": np.ascontiguousarray(sinc.T).astype(BF16),
            "cs": np.ascontiguousarray(cosc).astype(BF16),
            "ss": np.ascontiguousarray(sinc).astype(BF16),
            "wqT": np.ascontiguousarray(Wq[rs].T).astype(BF16),
            "wqmC": np.ascontiguousarray(Wq_mha[:, rs].T).astype(BF16),
            "wkmC": np.ascontiguousarray(Wk_mha[:, rs]).astype(BF16),
            "wk": np.ascontiguousarray(Wk[rs]).astype(BF16),
            "wvT": np.ascontiguousarray(Wv[rs].T).astype(BF16),
            "wvm": np.ascontiguousarray(Wv_mha[:, rs].T).astype(BF16),
            "woT": np.ascontiguousarray(Wo[rs].T).astype(BF16),
        }
        in_maps.append(m)

    global _last_in_maps, _last_res
    _last_in_maps = in_maps
    res = bass_utils.run_bass_kernel_spmd(nc, in_maps, core_ids=list(range(NC)))
    _last_res = res
    out = np.concatenate([np.asarray(res.results[c]["out"]).reshape(-1) for c in range(NC)])
    return out[None, :].astype(np.float32)



# revision 5
# speedup vs baseline: 1.0008x; 1.0008x over previous
"""Distributed Trainium2 Bass kernel for nn_Attention_74732430950409.

Single-query MHA with RoPE'd keys/values. All big GEMMs are folded onto the
tiny query/head side:

  qtil[h,:] = (((x @ Wq.T) @ Wq_mha.T)[h] @ Wk_mha[h]) @ Wk        (16, 2048)
  logits[s,h] = rope(keys)[s,:] . qtil[h,:] / sqrt(128)
  w = exp(logits)          (no max subtraction; |logits| < ~7)
  u[h,:] = sum_s w[s,h] * rope(states)[s,:]                        (16, 2048)
  z[h,:]  = (u[h,:] @ Wv.T) / l[h]
  attn[h,:] = z[h,:] @ Wv_mha[h].T
  out = attn.flat @ Wo.T + x

Key optimizations over the naive folded version:
  * RoPE's rotate-and-combine is folded into the PE: only the 4 elementwise
    products (a*cos, b*sin, b*cos, a*sin) are formed on DVE; the +- combine
    happens in PSUM accumulation using signed stationary vectors. 4 DVE ops
    per pair instead of 6, and no materialized roped tensors.
  * All projection weights are fp8(e4m3) scaled by 16; descaling is folded
    into existing activation scales (exp scale, z scale, attn copy scale).
  * Collective bounce copies ride the gpsimd SWDGE ring so they are not
    FIFO-blocked behind the bulk HBM stream on the sync HWDGE ring.
  * Bulk DMA issue order = dependency order: q-path weights, keys+tables,
    states+tables, epilogue weights. Collective payloads are bf16.

Sequence-sharded across 8 cores (1024 rows each). Four AllReduces:
AR(qh) 4KB, AR(qtil) 64KB, AR(u|l) 66KB, AR(attn) 4KB.
"""

import sys
import numpy as np

for p in ("/opt/trn_rl_repo",):
    if p not in sys.path:
        sys.path.insert(0, p)

import ml_dtypes

BF16 = ml_dtypes.bfloat16
FP8 = ml_dtypes.float8_e4m3fn

NUM_HEADS = 16
QK = 2048
VO = 2048
S = 8192
NC = 8
S_LOC = S // NC          # 1024
SH = VO // NC            # 256 rows per core of each weight
DQ = QK // NUM_HEADS     # 128
HALF = VO // 2           # 1024
ROPE_THETA = 10000.0
WSCALE = 16.0            # fp8 weight pre-scale (keeps values out of subnormals)

_cache = {}


def _build():
    import concourse.bass as bass
    import concourse.mybir as mybir
    import concourse.bacc as bacc
    import concourse.tile as tile

    f32 = mybir.dt.float32
    bf16 = mybir.dt.bfloat16
    fp8 = mybir.dt.float8e4
    AF = mybir.ActivationFunctionType
    ALU = mybir.AluOpType
    PSUM = bass.MemorySpace.PSUM

    nc = bacc.Bacc(None, target_bir_lowering=False)

    # ---------------- DRAM parameters (per-core shards, host pre-swizzled
    # so every DMA is a contiguous [128, X] block load) ----------------
    xq_d = nc.dram_tensor("xq", [128, 16], bf16, kind="ExternalInput")
    identb_d = nc.dram_tensor("identb", [16, 16], bf16, kind="ExternalInput")
    ident_d = nc.dram_tensor("ident", [16, 16], f32, kind="ExternalInput")
    xo_d = nc.dram_tensor("xo", [1, SH], f32, kind="ExternalInput")

    wqT_d = nc.dram_tensor("wqT", [128, 16, SH], fp8, kind="ExternalInput")
    wqmC_d = nc.dram_tensor("wqmC", [128, 2, QK], fp8, kind="ExternalInput")
    wkmC_d = nc.dram_tensor("wkmC", [128, 16, SH], fp8, kind="ExternalInput")
    wk_d = nc.dram_tensor("wk", [128, 2, VO], fp8, kind="ExternalInput")
    wvT_d = nc.dram_tensor("wvT", [128, 16, SH], fp8, kind="ExternalInput")
    wvm_d = nc.dram_tensor("wvm", [128, 2, VO], fp8, kind="ExternalInput")
    woT_d = nc.dram_tensor("woT", [128, 16, SH], fp8, kind="ExternalInput")

    keysT_d = nc.dram_tensor("keysT", [16, 128, S_LOC], bf16, kind="ExternalInput")
    ck_d = nc.dram_tensor("ck", [8, 128, S_LOC], bf16, kind="ExternalInput")
    sk_d = nc.dram_tensor("sk", [8, 128, S_LOC], bf16, kind="ExternalInput")
    st_d = nc.dram_tensor("st", [8, 128, VO], bf16, kind="ExternalInput")
    cs_d = nc.dram_tensor("cs", [8, 128, HALF], bf16, kind="ExternalInput")
    ss_d = nc.dram_tensor("ss", [8, 128, HALF], bf16, kind="ExternalInput")

    out_d = nc.dram_tensor("out", [1, SH], f32, kind="ExternalOutput")

    RG = [list(range(NC))]
    # exp scale: 1/sqrt(128) divided by the 16^4 carried by the 4 fp8 stages
    SCALE_EXP = float(1.0 / np.sqrt(DQ) / (WSCALE ** 4))

    with tile.TileContext(nc) as tc:
        with (
            tc.tile_pool(name="wts", bufs=1) as wts,
            tc.tile_pool(name="kbuf", bufs=1) as kbuf,
            tc.tile_pool(name="sbuf_s", bufs=1) as sbuf_s,
            tc.tile_pool(name="small", bufs=1) as small,
            tc.tile_pool(name="psA", bufs=4, space=PSUM) as psA,
            tc.tile_pool(name="psB", bufs=3, space=PSUM) as psB,
            tc.tile_pool(name="dram", bufs=1, space="DRAM") as dram,
        ):
            # ---------------- collective bounce buffers (DRAM) ----------------
            bqh_in = dram.tile([128, NUM_HEADS], bf16)
            bqh_out = dram.tile([128, NUM_HEADS], bf16)
            bqt_in = dram.tile([128, 16 * NUM_HEADS], bf16)
            bqt_out = dram.tile([128, 16 * NUM_HEADS], bf16)
            bu_in = dram.tile([128, 16 * NUM_HEADS + 1], bf16)
            bu_out = dram.tile([128, 16 * NUM_HEADS + 1], bf16)
            bat_in = dram.tile([128, NUM_HEADS], bf16)
            bat_out = dram.tile([128, NUM_HEADS], bf16)

            # ---------------- small persistent SBUF tiles ----------------
            x_sb = small.tile([128, 16], bf16, tag="x")
            identb = small.tile([16, 16], bf16, tag="idb")
            identf = small.tile([16, 16], f32, tag="idf")
            xo_sb = small.tile([1, SH], f32, tag="xo")
            qT_sb = small.tile([128, 2], bf16, tag="qT")
            qhTp_sb = small.tile([128, NUM_HEADS], bf16, tag="qhTp")
            qhT_sb = small.tile([128, NUM_HEADS], bf16, tag="qhT")
            tmpT_sb = small.tile([128, 2, NUM_HEADS], bf16, tag="tmpT")
            qtp_sb = small.tile([128, 16, NUM_HEADS], bf16, tag="qtp")
            qtilT_sb = small.tile([128, 16, NUM_HEADS], bf16, tag="qtilT")
            qtilTn_sb = small.tile([128, 8, NUM_HEADS], bf16, tag="qtilTn")
            w_sb = small.tile([NUM_HEADS, S_LOC], bf16, tag="w")
            l0_sb = small.tile([NUM_HEADS, 1], f32, tag="l0")
            l1_sb = small.tile([NUM_HEADS, 1], f32, tag="l1")
            lp_sb = small.tile([NUM_HEADS, 1], f32, tag="lp")
            wT_sb = small.tile([128, 8, NUM_HEADS], bf16, tag="wT")
            wTn_sb = small.tile([128, 8, NUM_HEADS], bf16, tag="wTn")
            u_sb = small.tile([NUM_HEADS, VO], f32, tag="u")
            ub_sb = small.tile([128, 16 * NUM_HEADS + 1], bf16, tag="ub")
            uT_bf = small.tile([128, 16, NUM_HEADS], bf16, tag="uTb")
            l_sb = small.tile([NUM_HEADS, 1], bf16, tag="l")
            l16_sb = small.tile([NUM_HEADS, 1], f32, tag="l16")
            rl_sb = small.tile([NUM_HEADS, 1], f32, tag="rl")
            z_sb = small.tile([NUM_HEADS, SH], bf16, tag="z")
            zT_sb = small.tile([128, 2, NUM_HEADS], bf16, tag="zT")
            atT_sb = small.tile([128, NUM_HEADS], bf16, tag="atT")
            atT_bf = small.tile([128, NUM_HEADS], bf16, tag="atTb")
            out_sb = small.tile([1, SH], f32, tag="out")

            # ================= DMA issue order on the sync HWDGE ring =======
            # 1) tiny parameters
            nc.sync.dma_start(x_sb[:], xq_d[:, :])
            nc.sync.dma_start(identb[:], identb_d[:, :])
            nc.sync.dma_start(identf[:], ident_d[:, :])
            nc.sync.dma_start(xo_sb[:], xo_d[:, :])

            # 2) q-path weights (gate the two early collectives)
            wqT_sb = wts.tile([128, 16, SH], fp8, tag="wqT")
            wqmC_sb = wts.tile([128, 2, QK], fp8, tag="wqmC")
            wkmC_sb = wts.tile([128, 16, SH], fp8, tag="wkmC")
            wk_sb = wts.tile([128, 2, VO], fp8, tag="wk")
            nc.sync.dma_start(wqT_sb[:], wqT_d[:, :, :])
            nc.sync.dma_start(wqmC_sb[:], wqmC_d[:, :, :])
            nc.sync.dma_start(wkmC_sb[:], wkmC_d[:, :, :])
            nc.sync.dma_start(wk_sb[:], wk_d[:, :, :])

            # 3) keys stream: per ci group {ck, sk, ktop, kbot}
            ck_t, sk_t, ka_t, kb_t = [], [], [], []
            for ci in range(8):
                ckt = kbuf.tile([128, S_LOC], bf16, tag="ck", bufs=8, name=f"ck{ci}")
                skt = kbuf.tile([128, S_LOC], bf16, tag="sk", bufs=4, name=f"sk{ci}")
                ka = kbuf.tile([128, S_LOC], bf16, tag="ka", bufs=8, name=f"ka{ci}")
                kb = kbuf.tile([128, S_LOC], bf16, tag="kb", bufs=4, name=f"kb{ci}")
                nc.sync.dma_start(ckt[:], ck_d[ci, :, :])
                nc.sync.dma_start(skt[:], sk_d[ci, :, :])
                nc.sync.dma_start(ka[:], keysT_d[ci, :, :])
                nc.sync.dma_start(kb[:], keysT_d[ci + 8, :, :])
                ck_t.append(ckt); sk_t.append(skt); ka_t.append(ka); kb_t.append(kb)

            # 4) states stream: per sb group {cs, ss, st}
            cs_t, ss_t, st_t = [], [], []
            for sb in range(8):
                cst = sbuf_s.tile([128, HALF], bf16, tag="cs", bufs=8, name=f"cs{sb}")
                sst = sbuf_s.tile([128, HALF], bf16, tag="ss", bufs=4, name=f"ss{sb}")
                stt = sbuf_s.tile([128, VO], bf16, tag="st", bufs=8, name=f"st{sb}")
                nc.sync.dma_start(cst[:], cs_d[sb, :, :])
                nc.sync.dma_start(sst[:], ss_d[sb, :, :])
                nc.sync.dma_start(stt[:], st_d[sb, :, :])
                cs_t.append(cst); ss_t.append(sst); st_t.append(stt)

            # 5) epilogue weights
            wvT_sb = wts.tile([128, 16, SH], fp8, tag="wvT")
            wvm_sb = wts.tile([128, 2, VO], fp8, tag="wvm")
            woT_sb = wts.tile([128, 16, SH], fp8, tag="woT")
            nc.sync.dma_start(wvT_sb[:], wvT_d[:, :, :])
            nc.sync.dma_start(wvm_sb[:], wvm_d[:, :, :])
            nc.sync.dma_start(woT_sb[:], woT_d[:, :, :])

            # ================= q path =================
            # qT = (x @ Wq.T)^T for the local 256-row shard, as [128, 2]
            for nc2 in range(2):
                qt_ps2 = psB.tile([128, 1], f32, tag="pB", name=f"qt_ps2_{nc2}")
                for kc in range(16):
                    nc.tensor.matmul(qt_ps2[:], wqT_sb[:, kc, nc2 * 128 : (nc2 + 1) * 128],
                                     x_sb[:, kc : kc + 1], start=(kc == 0), stop=(kc == 15))
                nc.scalar.activation(qT_sb[:, nc2 : nc2 + 1], qt_ps2[:], AF.Copy)

            # qh partial: (q_shard @ Wq_mha[:, shard].T)^T  -> AR#1
            qhT_ps = psB.tile([128, NUM_HEADS], f32, tag="pB")
            for h in range(NUM_HEADS):
                for nc2 in range(2):
                    nc.tensor.matmul(qhT_ps[:, h : h + 1],
                                     wqmC_sb[:, nc2, h * 128 : (h + 1) * 128],
                                     qT_sb[:, nc2 : nc2 + 1],
                                     start=(nc2 == 0), stop=(nc2 == 1))
            nc.scalar.activation(qhTp_sb[:], qhT_ps[:], AF.Copy)
            nc.gpsimd.dma_start(bqh_in[:], qhTp_sb[:])
            nc.gpsimd.collective_compute(
                "AllReduce", ALU.add, ins=[bqh_in[:].opt()], outs=[bqh_out[:].opt()],
                replica_groups=RG)
            nc.gpsimd.dma_start(qhT_sb[:], bqh_out[:, :])

            # tmpT[j, h] local j-shard
            tmpT_ps = [psB.tile([128, NUM_HEADS], f32, tag="pB", name=f"tmpT_ps{j}")
                       for j in range(2)]
            for h in range(NUM_HEADS):
                for jc in range(2):
                    nc.tensor.matmul(tmpT_ps[jc][:, h : h + 1],
                                     wkmC_sb[:, h, jc * 128 : (jc + 1) * 128],
                                     qhT_sb[:, h : h + 1], start=True, stop=True)
            for jc in range(2):
                nc.scalar.activation(tmpT_sb[:, jc, :], tmpT_ps[jc][:], AF.Copy)

            # qtil partial -> AR#2
            for ic in range(16):
                qt_ps = psB.tile([128, NUM_HEADS], f32, tag="pB")
                for jc in range(2):
                    nc.tensor.matmul(qt_ps[:], wk_sb[:, jc, ic * 128 : (ic + 1) * 128],
                                     tmpT_sb[:, jc, :], start=(jc == 0), stop=(jc == 1))
                nc.scalar.activation(qtp_sb[:, ic, :], qt_ps[:], AF.Copy)
            nc.gpsimd.dma_start(bqt_in[:, :].rearrange("p (ic h) -> p ic h", ic=16), qtp_sb[:])
            nc.gpsimd.collective_compute(
                "AllReduce", ALU.add, ins=[bqt_in[:].opt()], outs=[bqt_out[:].opt()],
                replica_groups=RG)
            nc.gpsimd.dma_start(
                qtilT_sb[:], bqt_out[:, :].rearrange("p (ic h) -> p ic h", ic=16))
            # negated first-half blocks (coefficient for the b*sin product)
            nc.scalar.activation(
                qtilTn_sb[:].rearrange("p a h -> p (a h)"),
                qtilT_sb[:, 0:8, :].rearrange("p a h -> p (a h)"),
                AF.Copy, scale=-1.0)

            # ================= keys: 4 rope products per pair tile =========
            # roped_top = a*ck - b*sk ; roped_bot = b*ck + a*sk
            # p1 = a*ck (new), p3 = b*ck (new), p4 = a*sk (into ck), p2 = b*sk (into a)
            p1_t, p2_t, p3_t, p4_t = [], [], [], []
            for ci in range(8):
                a, b = ka_t[ci], kb_t[ci]
                p1 = kbuf.tile([128, S_LOC], bf16, tag="p1", bufs=8, name=f"p1_{ci}")
                p3 = kbuf.tile([128, S_LOC], bf16, tag="p3", bufs=8, name=f"p3_{ci}")
                nc.vector.tensor_mul(p1[:], a[:], ck_t[ci][:])
                nc.vector.tensor_mul(p3[:], b[:], ck_t[ci][:])
                nc.vector.tensor_mul(ck_t[ci][:], a[:], sk_t[ci][:])   # p4
                nc.vector.tensor_mul(a[:], b[:], sk_t[ci][:])          # p2
                p1_t.append(p1); p3_t.append(p3); p4_t.append(ck_t[ci]); p2_t.append(a)

            # logits via signed PSUM accumulation (64 matmuls)
            lg_ps = [psA.tile([NUM_HEADS, 512], f32, tag="pA", name=f"lg{sc}")
                     for sc in range(2)]
            for ci in range(8):
                plist = [
                    (qtilT_sb[:, ci, :], p1_t[ci]),       # +qtil1 * a*ck
                    (qtilTn_sb[:, ci, :], p2_t[ci]),      # -qtil1 * b*sk
                    (qtilT_sb[:, 8 + ci, :], p3_t[ci]),   # +qtil2 * b*ck
                    (qtilT_sb[:, 8 + ci, :], p4_t[ci]),   # +qtil2 * a*sk
                ]
                for pi, (lhs, prod) in enumerate(plist):
                    for sc in range(2):
                        nc.tensor.matmul(
                            lg_ps[sc][:], lhs, prod[:, sc * 512 : (sc + 1) * 512],
                            start=(ci == 0 and pi == 0), stop=(ci == 7 and pi == 3))

            for sc in range(2):
                nc.scalar.activation(w_sb[:, sc * 512 : (sc + 1) * 512], lg_ps[sc][:],
                                     AF.Exp, scale=SCALE_EXP,
                                     accum_out=(l0_sb[:] if sc == 0 else l1_sb[:]))

            # wT via PE transpose, plus negated copy
            for sb in range(8):
                tr_ps = psB.tile([128, NUM_HEADS], bf16, tag="pB")
                nc.tensor.transpose(tr_ps[:], w_sb[:, sb * 128 : (sb + 1) * 128],
                                    identb[:, :])
                nc.scalar.activation(wT_sb[:, sb, :], tr_ps[:], AF.Copy)
                nc.scalar.activation(wTn_sb[:, sb, :], tr_ps[:], AF.Copy, scale=-1.0)

            # ================= states: 4 rope products per tile ============
            # roped_left = s1*cs - s2*ss ; roped_right = s2*cs + s1*ss
            # q1 = s1*cs (new), q3 = s2*cs (new), q4 = s1*ss (into cs), q2 = s2*ss (into s1)
            q1_t, q3_t = [], []
            for sb in range(8):
                stt = st_t[sb]
                q1 = sbuf_s.tile([128, HALF], bf16, tag="q1", bufs=6, name=f"q1_{sb}")
                q3 = sbuf_s.tile([128, HALF], bf16, tag="q3", bufs=6, name=f"q3_{sb}")
                nc.vector.tensor_mul(q1[:], stt[:, 0:HALF], cs_t[sb][:])
                nc.vector.tensor_mul(q3[:], stt[:, HALF:VO], cs_t[sb][:])
                nc.vector.tensor_mul(cs_t[sb][:], stt[:, 0:HALF], ss_t[sb][:])   # q4
                nc.vector.tensor_mul(stt[:, 0:HALF], stt[:, HALF:VO], ss_t[sb][:])  # q2
                q1_t.append(q1); q3_t.append(q3)

            nc.vector.tensor_add(lp_sb[:], l0_sb[:], l1_sb[:])

            # u via signed PSUM accumulation (64 matmuls), 4 column chunks
            u_ps = [psA.tile([NUM_HEADS, 512], f32, tag="pA", name=f"u_ps{i}")
                    for i in range(4)]
            for sb in range(8):
                wpos = wT_sb[:, sb, :]
                wneg = wTn_sb[:, sb, :]
                # u[:, 0:1024]    = sum wT*q1 - wT*q2   (q2 lives in st[:, 0:1024])
                # u[:, 1024:2048] = sum wT*q3 + wT*q4   (q4 lives in cs[sb])
                pieces = [
                    (u_ps[0], wpos, q1_t[sb][:, 0:512]),
                    (u_ps[0], wneg, st_t[sb][:, 0:512]),
                    (u_ps[1], wpos, q1_t[sb][:, 512:1024]),
                    (u_ps[1], wneg, st_t[sb][:, 512:1024]),
                    (u_ps[2], wpos, q3_t[sb][:, 0:512]),
                    (u_ps[2], wpos, cs_t[sb][:, 0:512]),
                    (u_ps[3], wpos, q3_t[sb][:, 512:1024]),
                    (u_ps[3], wpos, cs_t[sb][:, 512:1024]),
                ]
                for k, (ps, lhs, rhs) in enumerate(pieces):
                    nc.tensor.matmul(ps[:], lhs, rhs,
                                     start=(sb == 0 and k % 2 == 0),
                                     stop=(sb == 7 and k % 2 == 1))
            for nch in range(4):
                nc.scalar.activation(u_sb[:, nch * 512 : (nch + 1) * 512],
                                     u_ps[nch][:], AF.Copy)

            # uT via PE transpose -> bounce staging (bf16), l in last column
            for ic in range(16):
                tr_ps = psB.tile([128, NUM_HEADS], f32, tag="pB")
                nc.tensor.transpose(tr_ps[:], u_sb[:, ic * 128 : (ic + 1) * 128],
                                    identf[:, :])
                nc.scalar.activation(ub_sb[:, ic * 16 : (ic + 1) * 16], tr_ps[:], AF.Copy)
            nc.scalar.activation(ub_sb[0:NUM_HEADS, 256:257], lp_sb[:], AF.Copy)
            nc.gpsimd.dma_start(bu_in[:], ub_sb[:])
            nc.gpsimd.collective_compute(
                "AllReduce", ALU.add, ins=[bu_in[:].opt()], outs=[bu_out[:].opt()],
                replica_groups=RG)
            nc.gpsimd.dma_start(
                uT_bf[:], bu_out[:, 0:256].rearrange("p (ic h) -> p ic h", ic=16))
            nc.gpsimd.dma_start(l_sb[:], bu_out[0:NUM_HEADS, 256:257])
            nc.vector.tensor_scalar_mul(l16_sb[:], l_sb[:], WSCALE)
            nc.vector.reciprocal(rl_sb[:], l16_sb[:])

            # ================= epilogue =================
            z_ps = psB.tile([NUM_HEADS, SH], f32, tag="pB")
            for ic in range(16):
                nc.tensor.matmul(z_ps[:], uT_bf[:, ic, :], wvT_sb[:, ic, :],
                                 start=(ic == 0), stop=(ic == 15))
            nc.scalar.activation(z_sb[:], z_ps[:], AF.Copy, scale=rl_sb[:])

            for jc in range(2):
                tr_ps = psB.tile([128, NUM_HEADS], bf16, tag="pB")
                nc.tensor.transpose(tr_ps[:], z_sb[:, jc * 128 : (jc + 1) * 128],
                                    identb[:, :])
                nc.scalar.activation(zT_sb[:, jc, :], tr_ps[:], AF.Copy)

            at_ps = psB.tile([128, NUM_HEADS], f32, tag="pB")
            for h in range(NUM_HEADS):
                for jc in range(2):
                    nc.tensor.matmul(at_ps[:, h : h + 1],
                                     wvm_sb[:, jc, h * 128 : (h + 1) * 128],
                                     zT_sb[:, jc, h : h + 1],
                                     start=(jc == 0), stop=(jc == 1))
            # 1/256 descales wvm's x16 and pre-compensates woT's x16
            nc.scalar.activation(atT_sb[:], at_ps[:], AF.Copy,
                                 scale=float(1.0 / (WSCALE * WSCALE)))
            nc.gpsimd.dma_start(bat_in[:], atT_sb[:])
            nc.gpsimd.collective_compute(
                "AllReduce", ALU.add, ins=[bat_in[:].opt()], outs=[bat_out[:].opt()],
                replica_groups=RG)
            nc.gpsimd.dma_start(atT_bf[:], bat_out[:, :])

            o_ps = psB.tile([1, SH], f32, tag="pB")
            for h in range(NUM_HEADS):
                nc.tensor.matmul(o_ps[:], atT_bf[:, h : h + 1], woT_sb[:, h, :],
                                 start=(h == 0), stop=(h == NUM_HEADS - 1))
            nc.vector.tensor_add(out_sb[:], o_ps[:], xo_sb[:])
            nc.sync.dma_start(out_d[:, :], out_sb[:])

    nc.compile()
    return nc


def _tables():
    half = HALF
    freqs = 1.0 / (ROPE_THETA ** (np.arange(half, dtype=np.float32) * 2.0 / VO))
    ang = np.outer(np.arange(S, dtype=np.float32), freqs).astype(np.float32)  # (S, half)
    return np.cos(ang), np.sin(ang)


def _w8(a):
    # fp8 weights pre-scaled by 16 to stay in e4m3's normal range
    return np.ascontiguousarray(a * WSCALE).astype(FP8)


def _swz(a, tiles):
    # [tiles*128, X] -> [128, tiles, X] so each partition's data is contiguous
    X = a.shape[1]
    return np.ascontiguousarray(a.reshape(tiles, 128, X).transpose(1, 0, 2))


def kernel(x, keys, states, Wq, Wk, Wv, Wq_mha, Wk_mha, Wv_mha, Wo):
    from concourse import bass_utils

    if "nc" not in _cache:
        _cache["nc"] = _build()
    nc = _cache["nc"]

    x = np.asarray(x, np.float32)
    keys = np.asarray(keys, np.float32)
    states = np.asarray(states, np.float32)
    cos_t, sin_t = _tables()

    xq2d = np.ascontiguousarray(x.reshape(16, 128).T).astype(BF16)
    ident16b = np.eye(16, dtype=np.float32).astype(BF16)
    ident16f = np.eye(16, dtype=np.float32)

    in_maps = []
    for c in range(NC):
        rs = slice(c * SH, (c + 1) * SH)
        ssl = slice(c * S_LOC, (c + 1) * S_LOC)
        cosc = cos_t[ssl]            # (1024, 1024) [s_loc, j]
        sinc = sin_t[ssl]
        m = {
            "xq": xq2d,
            "identb": ident16b,
            "ident": ident16f,
            "xo": np.ascontiguousarray(x[rs]).reshape(1, SH),
            "wqT": _swz(_w8(Wq[rs].T), 16).reshape(128, 16, SH),
            "wqmC": _swz(_w8(Wq_mha[:, rs].T), 2).reshape(128, 2, QK),
            "wkmC": _swz(_w8(Wk_mha[:, rs]), 16).reshape(128, 16, SH),
            "wk": _swz(_w8(Wk[rs]), 2).reshape(128, 2, VO),
            "wvT": _swz(_w8(Wv[rs].T), 16).reshape(128, 16, SH),
            "wvm": _swz(_w8(Wv_mha[:, rs].T), 2).reshape(128, 2, VO),
            "woT": _swz(_w8(Wo[rs].T), 16).reshape(128, 16, SH),
            "keysT": np.ascontiguousarray(keys[ssl].T).astype(BF16)
                .reshape(16, 128, S_LOC),
            "ck": np.ascontiguousarray(cosc.T).astype(BF16).reshape(8, 128, S_LOC),
            "sk": np.ascontiguousarray(sinc.T).astype(BF16).reshape(8, 128, S_LOC),
            "st": np.ascontiguousarray(states[ssl]).astype(BF16).reshape(8, 128, VO),
            "cs": np.ascontiguousarray(cosc).astype(BF16).reshape(8, 128, HALF),
            "ss": np.ascontiguousarray(sinc).astype(BF16).reshape(8, 128, HALF),
        }
        in_maps.append(m)

    global _last_in_maps, _last_res
    _last_in_maps = in_maps
    res = bass_utils.run_bass_kernel_spmd(nc, in_maps, core_ids=list(range(NC)))
    _last_res = res
    out = np.concatenate([np.asarray(res.results[c]["out"]).reshape(-1) for c in range(NC)])
    return out[None, :].astype(np.float32)


# revision 16
# speedup vs baseline: 1.1359x; 1.1349x over previous
"""Distributed Trainium2 Bass kernel for nn_Attention_74732430950409.

Single-query MHA with RoPE'd keys/values. All big GEMMs are folded onto the
tiny query/head side:

  qtil[h,:] = (((x @ Wq.T) @ Wq_mha.T)[h] @ Wk_mha[h]) @ Wk        (16, 2048)
  logits[s,h] = rope(keys)[s,:] . qtil[h,:] / sqrt(128)
  w = exp(logits)          (no max subtraction; |logits| < ~7)
  u[h,:] = sum_s w[s,h] * rope(states)[s,:]                        (16, 2048)
  z[h,:]  = (u[h,:] @ Wv.T) / l[h]
  attn[h,:] = z[h,:] @ Wv_mha[h].T
  out = attn.flat @ Wo.T + x

Timing model (measured): the 8 SPMD cores launch ~60us apart, so the first
collective is a rendezvous that dominates the front half. Everything before
it (bulk DMA, RoPE on DVE, q-path) is effectively free; the optimization
target is the post-rendezvous serial chain AR(qh) -> AR(qtil) -> logits ->
u -> AR(u|l) -> epilogue -> AG(attn) -> out. Hence:
  * classic 6-op RoPE on DVE (hidden pre-rendezvous) to halve the
    post-rendezvous PE matmul count,
  * fp8(e4m3, x16-prescaled) weights; descales folded into activation
    scales,
  * merged DMA transfers (the sync sequencer pays ~0.6us dispatch per DMA),
  * collective bounces on the gpsimd SWDGE ring (not FIFO-blocked behind
    bulk HBM traffic),
  * bf16 collective payloads; final collective is an AllGather + local sum
    (AG floor ~4.6us vs AR ~10us),
  * PSUM->SBUF copies bundled into few wide activations.
"""

import sys
import numpy as np

for p in ("/opt/trn_rl_repo",):
    if p not in sys.path:
        sys.path.insert(0, p)

import ml_dtypes

BF16 = ml_dtypes.bfloat16
FP8 = ml_dtypes.float8_e4m3fn

NUM_HEADS = 16
QK = 2048
VO = 2048
S = 8192
NC = 8
S_LOC = S // NC          # 1024
SH = VO // NC            # 256 rows per core of each weight
DQ = QK // NUM_HEADS     # 128
HALF = VO // 2           # 1024
ROPE_THETA = 10000.0
WSCALE = 16.0            # fp8 weight pre-scale (keeps values out of subnormals)

_cache = {}


def _build():
    import concourse.bass as bass
    import concourse.mybir as mybir
    import concourse.bacc as bacc
    import concourse.tile as tile

    f32 = mybir.dt.float32
    bf16 = mybir.dt.bfloat16
    fp8 = mybir.dt.float8e4
    AF = mybir.ActivationFunctionType
    ALU = mybir.AluOpType
    PSUM = bass.MemorySpace.PSUM

    nc = bacc.Bacc(None, target_bir_lowering=False)

    # ---------------- DRAM parameters (host pre-swizzled; contiguous) ------
    xq_d = nc.dram_tensor("xq", [128, 16], bf16, kind="ExternalInput")
    identb_d = nc.dram_tensor("identb", [128, 128], bf16, kind="ExternalInput")
    identf_d = nc.dram_tensor("identf", [16, 16], f32, kind="ExternalInput")
    xo_d = nc.dram_tensor("xo", [1, SH], f32, kind="ExternalInput")

    wq4_d = nc.dram_tensor("wq4", [128, 4, 4096], fp8, kind="ExternalInput")
    wep_d = nc.dram_tensor("wep", [128, 3, 4096], fp8, kind="ExternalInput")

    kab_d = nc.dram_tensor("kab", [8, 128, 2, S_LOC], bf16, kind="ExternalInput")
    cksk_d = nc.dram_tensor("cksk", [8, 128, 2, S_LOC], bf16, kind="ExternalInput")
    st_d = nc.dram_tensor("st", [8, 128, VO], bf16, kind="ExternalInput")
    csss_d = nc.dram_tensor("csss", [8, 128, 2, HALF], bf16, kind="ExternalInput")

    out_d = nc.dram_tensor("out", [1, SH], f32, kind="ExternalOutput")

    RG = [list(range(NC))]
    SCALE_EXP = float(1.0 / np.sqrt(DQ) / (WSCALE ** 4))

    with tile.TileContext(nc) as tc:
        with (
            tc.tile_pool(name="wts", bufs=1) as wts,
            tc.tile_pool(name="kbuf", bufs=1) as kbuf,
            tc.tile_pool(name="sbuf_s", bufs=1) as sbuf_s,
            tc.tile_pool(name="tmps", bufs=1) as tmps,
            tc.tile_pool(name="small", bufs=1) as small,
            tc.tile_pool(name="psA", bufs=4, space=PSUM) as psA,
            tc.tile_pool(name="psB", bufs=4, space=PSUM) as psB,
            tc.tile_pool(name="dram", bufs=1, space="DRAM") as dram,
        ):
            # ---------------- collective bounce buffers (DRAM) ----------------
            bqh_in = dram.tile([128, NUM_HEADS], bf16)
            bqh_out = dram.tile([128, NUM_HEADS], bf16)
            bqt_in = dram.tile([128, 16 * NUM_HEADS], bf16)
            bqt_out = dram.tile([128, 16 * NUM_HEADS], bf16)
            bu_in = dram.tile([128, 16 * NUM_HEADS + 1], bf16)
            bu_out = dram.tile([128, 16 * NUM_HEADS + 1], bf16)
            bat_in = dram.tile([NUM_HEADS, 128], bf16)
            bat_out = dram.tile([128, 128], bf16)

            # ---------------- small persistent SBUF tiles ----------------
            x_sb = small.tile([128, 16], bf16, tag="x")
            identb = small.tile([128, 128], bf16, tag="idb")
            identf = small.tile([16, 16], f32, tag="idf")
            xo_sb = small.tile([1, SH], f32, tag="xo")
            qT_sb = small.tile([128, 2], bf16, tag="qT")
            qhTp_sb = small.tile([128, NUM_HEADS], bf16, tag="qhTp")
            qhT_sb = small.tile([128, NUM_HEADS], bf16, tag="qhT")
            tmpT_sb = small.tile([128, 2, NUM_HEADS], bf16, tag="tmpT")
            qtp_sb = small.tile([128, 16 * NUM_HEADS], bf16, tag="qtp")
            qtilT_sb = small.tile([128, 16, NUM_HEADS], bf16, tag="qtilT")
            w_sb = small.tile([NUM_HEADS, S_LOC], bf16, tag="w")
            l0_sb = small.tile([NUM_HEADS, 1], f32, tag="l0")
            l1_sb = small.tile([NUM_HEADS, 1], f32, tag="l1")
            lp_sb = small.tile([NUM_HEADS, 1], f32, tag="lp")
            wT_sb = small.tile([128, 8, NUM_HEADS], bf16, tag="wT")
            u_sb = small.tile([NUM_HEADS, VO], f32, tag="u")
            ub_sb = small.tile([128, 16 * NUM_HEADS + 1], bf16, tag="ub")
            uT_bf = small.tile([128, 16, NUM_HEADS], bf16, tag="uTb")
            l_sb = small.tile([NUM_HEADS, 1], bf16, tag="l")
            l16_sb = small.tile([NUM_HEADS, 1], f32, tag="l16")
            rl_sb = small.tile([NUM_HEADS, 1], f32, tag="rl")
            z_sb = small.tile([NUM_HEADS, SH], bf16, tag="z")
            zT_sb = small.tile([128, 2, NUM_HEADS], bf16, tag="zT")
            atT_sb = small.tile([128, NUM_HEADS], bf16, tag="atT")
            atr_sb = small.tile([NUM_HEADS, 128], bf16, tag="atr")
            aga_sb = small.tile([128, 128], bf16, tag="aga")
            agat_sb = small.tile([128, 128], bf16, tag="agat")
            atT_bf = small.tile([128, NUM_HEADS], bf16, tag="atTb")
            out_sb = small.tile([1, SH], f32, tag="out")

            # ================= DMA issue order (sync HWDGE ring) ===========
            nc.sync.dma_start(x_sb[:], xq_d[:, :])
            nc.sync.dma_start(identb[:], identb_d[:, :])
            nc.sync.dma_start(identf[:], identf_d[:, :])
            nc.sync.dma_start(xo_sb[:], xo_d[:, :])

            wq4_sb = wts.tile([128, 4, 4096], fp8, tag="wq4")
            nc.sync.dma_start(wq4_sb[:], wq4_d[:, :, :])

            kab_t, cksk_t = [], []
            for ci in range(8):
                kab = kbuf.tile([128, 2, S_LOC], bf16, tag="kab", bufs=8, name=f"kab{ci}")
                cksk = kbuf.tile([128, 2, S_LOC], bf16, tag="cksk", bufs=4,
                                 name=f"cksk{ci}")
                nc.sync.dma_start(kab[:], kab_d[ci, :, :, :])
                nc.sync.dma_start(cksk[:], cksk_d[ci, :, :, :])
                kab_t.append(kab); cksk_t.append(cksk)

            st_t, csss_t = [], []
            for sb in range(8):
                stt = sbuf_s.tile([128, VO], bf16, tag="st", bufs=8, name=f"st{sb}")
                csss = sbuf_s.tile([128, 2, HALF], bf16, tag="csss", bufs=4,
                                   name=f"csss{sb}")
                nc.sync.dma_start(stt[:], st_d[sb, :, :])
                nc.sync.dma_start(csss[:], csss_d[sb, :, :, :])
                st_t.append(stt); csss_t.append(csss)

            wep_sb = wts.tile([128, 3, 4096], fp8, tag="wep")
            nc.sync.dma_start(wep_sb[:], wep_d[:, :, :])

            # weight slice helpers (flat fp8 packs)
            wqT_s = lambda kc, lo, hi: wq4_sb[:, 0, kc * 256 + lo : kc * 256 + hi]
            wqm_s = lambda n2, lo, hi: wq4_sb[:, 1, n2 * 2048 + lo : n2 * 2048 + hi]
            wkm_s = lambda h, lo, hi: wq4_sb[:, 2, h * 256 + lo : h * 256 + hi]
            wk_s = lambda jc, lo, hi: wq4_sb[:, 3, jc * 2048 + lo : jc * 2048 + hi]
            wvT_s = lambda ic, lo, hi: wep_sb[:, 0, ic * 256 + lo : ic * 256 + hi]
            wvm_s = lambda jc, lo, hi: wep_sb[:, 1, jc * 2048 + lo : jc * 2048 + hi]
            woT_s = lambda h, lo, hi: wep_sb[:, 2, h * 256 + lo : h * 256 + hi]

            # ================= q path =================
            for nc2 in range(2):
                qt_ps2 = psB.tile([128, 1], f32, tag="pB", name=f"qt_ps2_{nc2}")
                for kc in range(16):
                    nc.tensor.matmul(qt_ps2[:], wqT_s(kc, nc2 * 128, (nc2 + 1) * 128),
                                     x_sb[:, kc : kc + 1], start=(kc == 0), stop=(kc == 15))
                nc.scalar.activation(qT_sb[:, nc2 : nc2 + 1], qt_ps2[:], AF.Copy)

            qhT_ps = psB.tile([128, NUM_HEADS], f32, tag="pB")
            for h in range(NUM_HEADS):
                for nc2 in range(2):
                    nc.tensor.matmul(qhT_ps[:, h : h + 1],
                                     wqm_s(nc2, h * 128, (h + 1) * 128),
                                     qT_sb[:, nc2 : nc2 + 1],
                                     start=(nc2 == 0), stop=(nc2 == 1))
            nc.scalar.activation(qhTp_sb[:], qhT_ps[:], AF.Copy)
            nc.gpsimd.dma_start(bqh_in[:], qhTp_sb[:])
            nc.gpsimd.collective_compute(
                "AllReduce", ALU.add, ins=[bqh_in[:].opt()], outs=[bqh_out[:].opt()],
                replica_groups=RG)
            nc.gpsimd.dma_start(qhT_sb[:], bqh_out[:, :])

            tmpT_ps = [psB.tile([128, NUM_HEADS], f32, tag="pB", name=f"tmpT_ps{j}")
                       for j in range(2)]
            for h in range(NUM_HEADS):
                for jc in range(2):
                    nc.tensor.matmul(tmpT_ps[jc][:, h : h + 1],
                                     wkm_s(h, jc * 128, (jc + 1) * 128),
                                     qhT_sb[:, h : h + 1], start=True, stop=True)
            for jc in range(2):
                nc.scalar.activation(tmpT_sb[:, jc, :], tmpT_ps[jc][:], AF.Copy)

            # qtil partial into two wide PSUM tiles, two wide copies
            qt_big = [psB.tile([128, 128], f32, tag="pB", name=f"qt_big{g}")
                      for g in range(2)]
            for ic in range(16):
                g, col = ic // 8, (ic % 8) * 16
                for jc in range(2):
                    nc.tensor.matmul(qt_big[g][:, col : col + 16],
                                     wk_s(jc, ic * 128, (ic + 1) * 128),
                                     tmpT_sb[:, jc, :], start=(jc == 0), stop=(jc == 1))
            for g in range(2):
                nc.scalar.activation(qtp_sb[:, g * 128 : (g + 1) * 128], qt_big[g][:],
                                     AF.Copy)
            nc.gpsimd.dma_start(bqt_in[:, :], qtp_sb[:])
            nc.gpsimd.collective_compute(
                "AllReduce", ALU.add, ins=[bqt_in[:].opt()], outs=[bqt_out[:].opt()],
                replica_groups=RG)
            nc.gpsimd.dma_start(
                qtilT_sb[:], bqt_out[:, :].rearrange("p (ic h) -> p ic h", ic=16))

            # ================= keys rope (classic, on DVE, pre-rendezvous) ==
            for ci in range(8):
                a = kab_t[ci][:, 0, :]
                b = kab_t[ci][:, 1, :]
                c = cksk_t[ci][:, 0, :]
                s = cksk_t[ci][:, 1, :]
                t1 = tmps.tile([128, S_LOC], bf16, tag="rt", bufs=8)
                t2 = tmps.tile([128, S_LOC], bf16, tag="rt", bufs=8)
                t3 = tmps.tile([128, S_LOC], bf16, tag="rt", bufs=8)
                t4 = tmps.tile([128, S_LOC], bf16, tag="rt", bufs=8)
                nc.vector.tensor_mul(t1[:], a, c)
                nc.vector.tensor_mul(t2[:], b, s)
                nc.vector.tensor_mul(t3[:], b, c)
                nc.vector.tensor_mul(t4[:], a, s)
                nc.vector.tensor_sub(a, t1[:], t2[:])
                nc.vector.tensor_add(b, t3[:], t4[:])

            # logits: 32 matmuls into two PSUM chunks
            lg_ps = [psA.tile([NUM_HEADS, 512], f32, tag="pA", name=f"lg{sc}")
                     for sc in range(2)]
            for ci in range(8):
                for hf in range(2):
                    lhs = qtilT_sb[:, 8 * hf + ci, :]
                    for sc in range(2):
                        nc.tensor.matmul(
                            lg_ps[sc][:], lhs,
                            kab_t[ci][:, hf, sc * 512 : (sc + 1) * 512],
                            start=(ci == 0 and hf == 0), stop=(ci == 7 and hf == 1))

            # ================= states rope (classic, pre-rendezvous) ========
            for sb in range(8):
                stt = st_t[sb]
                c = csss_t[sb][:, 0, :]
                s = csss_t[sb][:, 1, :]
                t1 = tmps.tile([128, HALF], bf16, tag="rt", bufs=8)
                t2 = tmps.tile([128, HALF], bf16, tag="rt", bufs=8)
                t3 = tmps.tile([128, HALF], bf16, tag="rt", bufs=8)
                t4 = tmps.tile([128, HALF], bf16, tag="rt", bufs=8)
                nc.vector.tensor_mul(t1[:], stt[:, 0:HALF], c)
                nc.vector.tensor_mul(t2[:], stt[:, HALF:VO], s)
                nc.vector.tensor_mul(t3[:], stt[:, HALF:VO], c)
                nc.vector.tensor_mul(t4[:], stt[:, 0:HALF], s)
                nc.vector.tensor_sub(stt[:, 0:HALF], t1[:], t2[:])
                nc.vector.tensor_add(stt[:, HALF:VO], t3[:], t4[:])

            # exp + l, interleaved with wT transposes (PE) and bundled copies
            wt_ps = [psB.tile([128, 4 * NUM_HEADS], bf16, tag="pB", name=f"wt_ps{g}")
                     for g in range(2)]
            for sc in range(2):
                nc.scalar.activation(w_sb[:, sc * 512 : (sc + 1) * 512], lg_ps[sc][:],
                                     AF.Exp, scale=SCALE_EXP,
                                     accum_out=(l0_sb[:] if sc == 0 else l1_sb[:]))
                for k in range(4):
                    sb = sc * 4 + k
                    nc.tensor.transpose(wt_ps[sc][:, k * 16 : (k + 1) * 16],
                                        w_sb[:, sb * 128 : (sb + 1) * 128],
                                        identb[0:16, 0:16])
                nc.scalar.activation(
                    wT_sb[:, 4 * sc : 4 * (sc + 1), :].rearrange("p a h -> p (a h)"),
                    wt_ps[sc][:], AF.Copy)
            nc.vector.tensor_add(lp_sb[:], l0_sb[:], l1_sb[:])

            # u: 32 matmuls into four PSUM chunks
            u_ps = [psA.tile([NUM_HEADS, 512], f32, tag="pA", name=f"u_ps{i}")
                    for i in range(4)]
            for sb in range(8):
                for nch in range(4):
                    nc.tensor.matmul(u_ps[nch][:], wT_sb[:, sb, :],
                                     st_t[sb][:, nch * 512 : (nch + 1) * 512],
                                     start=(sb == 0), stop=(sb == 7))
            for nch in range(4):
                if nch % 2 == 0:
                    nc.scalar.activation(u_sb[:, nch * 512 : (nch + 1) * 512],
                                         u_ps[nch][:], AF.Copy)
                else:
                    nc.vector.tensor_copy(u_sb[:, nch * 512 : (nch + 1) * 512],
                                          u_ps[nch][:])

            # uT via PE transposes, bundled into 4 wide copies
            ut_ps = [psB.tile([128, 4 * NUM_HEADS], f32, tag="pB", name=f"ut_ps{g}")
                     for g in range(4)]
            for g in range(4):
                for k in range(4):
                    ic = g * 4 + k
                    nc.tensor.transpose(ut_ps[g][:, k * 16 : (k + 1) * 16],
                                        u_sb[:, ic * 128 : (ic + 1) * 128],
                                        identf[:, :])
                nc.scalar.activation(ub_sb[:, g * 64 : (g + 1) * 64], ut_ps[g][:],
                                     AF.Copy)
            nc.scalar.activation(ub_sb[0:NUM_HEADS, 256:257], lp_sb[:], AF.Copy)
            nc.gpsimd.dma_start(bu_in[:], ub_sb[:])
            nc.gpsimd.collective_compute(
                "AllReduce", ALU.add, ins=[bu_in[:].opt()], outs=[bu_out[:].opt()],
                replica_groups=RG)
            nc.gpsimd.dma_start(
                uT_bf[:], bu_out[:, 0:256].rearrange("p (ic h) -> p ic h", ic=16))
            nc.gpsimd.dma_start(l_sb[:], bu_out[0:NUM_HEADS, 256:257])
            nc.vector.tensor_scalar_mul(l16_sb[:], l_sb[:], WSCALE)
            nc.vector.reciprocal(rl_sb[:], l16_sb[:])

            # ================= epilogue =================
            z_ps = psB.tile([NUM_HEADS, SH], f32, tag="pB")
            for ic in range(16):
                nc.tensor.matmul(z_ps[:], uT_bf[:, ic, :], wvT_s(ic, 0, 256),
                                 start=(ic == 0), stop=(ic == 15))
            nc.scalar.activation(z_sb[:], z_ps[:], AF.Copy, scale=rl_sb[:])

            zt_ps = psB.tile([128, 2 * NUM_HEADS], bf16, tag="pB")
            for jc in range(2):
                nc.tensor.transpose(zt_ps[:, jc * 16 : (jc + 1) * 16],
                                    z_sb[:, jc * 128 : (jc + 1) * 128],
                                    identb[0:16, 0:16])
            nc.scalar.activation(zT_sb[:].rearrange("p a h -> p (a h)"), zt_ps[:],
                                 AF.Copy)

            at_ps = psB.tile([128, NUM_HEADS], f32, tag="pB")
            for h in range(NUM_HEADS):
                for jc in range(2):
                    nc.tensor.matmul(at_ps[:, h : h + 1],
                                     wvm_s(jc, h * 128, (h + 1) * 128),
                                     zT_sb[:, jc, h : h + 1],
                                     start=(jc == 0), stop=(jc == 1))
            # 1/256 descales wvm's x16 and pre-compensates woT's x16
            nc.scalar.activation(atT_sb[:], at_ps[:], AF.Copy,
                                 scale=float(1.0 / (WSCALE * WSCALE)))
            # transpose to [16, 128] for the partition-axis AllGather
            atr_ps = psB.tile([NUM_HEADS, 128], bf16, tag="pB")
            nc.tensor.transpose(atr_ps[:], atT_sb[:], identb[:, :])
            nc.scalar.activation(atr_sb[:], atr_ps[:], AF.Copy)
            nc.gpsimd.dma_start(bat_in[:], atr_sb[:])
            nc.gpsimd.collective_compute(
                "AllGather", ALU.bypass, ins=[bat_in[:].opt()], outs=[bat_out[:].opt()],
                replica_groups=RG)
            nc.gpsimd.dma_start(aga_sb[:], bat_out[:, :])
            # transpose the gathered [16r+h, d] blocks to [d, 16r+h], then the
            # 8 rank blocks become free-dim slices we can sum on DVE
            agat_ps = psB.tile([128, 128], bf16, tag="pB")
            nc.tensor.transpose(agat_ps[:], aga_sb[:], identb[:, :])
            nc.scalar.activation(agat_sb[:], agat_ps[:], AF.Copy)
            nc.vector.tensor_add(atT_bf[:], agat_sb[:, 0:16], agat_sb[:, 16:32])
            for r in range(2, 8):
                nc.vector.tensor_add(atT_bf[:], atT_bf[:],
                                     agat_sb[:, 16 * r : 16 * (r + 1)])

            o_ps = psB.tile([1, SH], f32, tag="pB")
            for h in range(NUM_HEADS):
                nc.tensor.matmul(o_ps[:], atT_bf[:, h : h + 1], woT_s(h, 0, 256),
                                 start=(h == 0), stop=(h == NUM_HEADS - 1))
            nc.vector.tensor_add(out_sb[:], o_ps[:], xo_sb[:])
            nc.sync.dma_start(out_d[:, :], out_sb[:])

    nc.compile()
    return nc


def _tables():
    half = HALF
    freqs = 1.0 / (ROPE_THETA ** (np.arange(half, dtype=np.float32) * 2.0 / VO))
    ang = np.outer(np.arange(S, dtype=np.float32), freqs).astype(np.float32)  # (S, half)
    return np.cos(ang), np.sin(ang)


def _w8flat(a, tiles):
    # [tiles*128, X] fp8-prescaled -> [128, tiles*X] (partition-contiguous)
    X = a.shape[1]
    sw = np.ascontiguousarray(
        (np.asarray(a, np.float32) * WSCALE).reshape(tiles, 128, X).transpose(1, 0, 2))
    return sw.reshape(128, tiles * X).astype(FP8)


def kernel(x, keys, states, Wq, Wk, Wv, Wq_mha, Wk_mha, Wv_mha, Wo):
    from concourse import bass_utils

    if "nc" not in _cache:
        _cache["nc"] = _build()
    nc = _cache["nc"]

    x = np.asarray(x, np.float32)
    keys = np.asarray(keys, np.float32)
    states = np.asarray(states, np.float32)
    cos_t, sin_t = _tables()

    xq2d = np.ascontiguousarray(x.reshape(16, 128).T).astype(BF16)
    ident128b = np.eye(128, dtype=np.float32).astype(BF16)
    ident16f = np.eye(16, dtype=np.float32)

    in_maps = []
    for c in range(NC):
        rs = slice(c * SH, (c + 1) * SH)
        ssl = slice(c * S_LOC, (c + 1) * S_LOC)
        cosc = cos_t[ssl]            # (1024, 1024) [s_loc, j]
        sinc = sin_t[ssl]

        wq4 = np.stack([
            _w8flat(Wq[rs].T, 16),
            _w8flat(Wq_mha[:, rs].T, 2),
            _w8flat(Wk_mha[:, rs], 16),
            _w8flat(Wk[rs], 2),
        ], axis=1)                   # [128, 4, 4096]
        wep = np.stack([
            _w8flat(Wv[rs].T, 16),
            _w8flat(Wv_mha[:, rs].T, 2),
            _w8flat(Wo[rs].T, 16),
        ], axis=1)                   # [128, 3, 4096]

        kT = keys[ssl].T.astype(BF16).reshape(16, 128, S_LOC)
        kab = np.stack([kT[0:8], kT[8:16]], axis=2)          # [8, 128, 2, 1024]
        cT = cosc.T.astype(BF16).reshape(8, 128, S_LOC)
        sT = sinc.T.astype(BF16).reshape(8, 128, S_LOC)
        cksk = np.stack([cT, sT], axis=2)                    # [8, 128, 2, 1024]
        cN = cosc.astype(BF16).reshape(8, 128, HALF)
        sN = sinc.astype(BF16).reshape(8, 128, HALF)
        csss = np.stack([cN, sN], axis=2)                    # [8, 128, 2, 1024]

        m = {
            "xq": xq2d,
            "identb": ident128b,
            "identf": ident16f,
            "xo": np.ascontiguousarray(x[rs]).reshape(1, SH),
            "wq4": np.ascontiguousarray(wq4),
            "wep": np.ascontiguousarray(wep),
            "kab": np.ascontiguousarray(kab),
            "cksk": np.ascontiguousarray(cksk),
            "st": np.ascontiguousarray(states[ssl]).astype(BF16).reshape(8, 128, VO),
            "csss": np.ascontiguousarray(csss),
        }
        in_maps.append(m)

    global _last_in_maps, _last_res
    _last_in_maps = in_maps
    res = bass_utils.run_bass_kernel_spmd(nc, in_maps, core_ids=list(range(NC)))
    _last_res = res
    out = np.concatenate([np.asarray(res.results[c]["out"]).reshape(-1) for c in range(NC)])
    return out[None, :].astype(np.float32)


# revision 24
# speedup vs baseline: 1.1986x; 1.0552x over previous
"""Distributed Trainium2 Bass kernel for nn_Attention_74732430950409.

Single-query MHA with RoPE'd keys/values. All big GEMMs are folded onto the
tiny query/head side:

  qtil[h,:] = (((x @ Wq.T) @ Wq_mha.T)[h] @ Wk_mha[h]) @ Wk        (16, 2048)
  logits[s,h] = rope(keys)[s,:] . qtil[h,:] / sqrt(128)
  w = exp(logits)          (no max subtraction; |logits| < ~7)
  u[h,:] = sum_s w[s,h] * rope(states)[s,:]                        (16, 2048)
  z[h,:]  = (u[h,:] @ Wv.T) / l[h]
  attn[h,:] = z[h,:] @ Wv_mha[h].T
  out = attn.flat @ Wo.T + x

Timing model (measured): the 8 SPMD cores launch ~60us apart, so the first
collective is a rendezvous that dominates the front half. Everything before
it (bulk DMA, RoPE on DVE, q-path) is effectively free; the optimization
target is the post-rendezvous serial chain AR(qh) -> AR(qtil) -> logits ->
u -> AR(u|l) -> epilogue -> AG(attn) -> out. Hence:
  * classic 6-op RoPE on DVE (hidden pre-rendezvous) to halve the
    post-rendezvous PE matmul count,
  * fp8(e4m3, x16-prescaled) weights; descales folded into activation
    scales,
  * merged DMA transfers (the sync sequencer pays ~0.6us dispatch per DMA),
  * collective bounces on the gpsimd SWDGE ring (not FIFO-blocked behind
    bulk HBM traffic),
  * bf16 collective payloads; final collective is an AllGather + local sum
    (AG floor ~4.6us vs AR ~10us),
  * PSUM->SBUF copies bundled into few wide activations.
"""

import sys
import numpy as np

for p in ("/opt/trn_rl_repo",):
    if p not in sys.path:
        sys.path.insert(0, p)

import ml_dtypes

BF16 = ml_dtypes.bfloat16
FP8 = ml_dtypes.float8_e4m3fn

NUM_HEADS = 16
QK = 2048
VO = 2048
S = 8192
NC = 8
S_LOC = S // NC          # 1024
SH = VO // NC            # 256 rows per core of each weight
DQ = QK // NUM_HEADS     # 128
HALF = VO // 2           # 1024
ROPE_THETA = 10000.0
WSCALE = 16.0            # fp8 weight pre-scale (keeps values out of subnormals)

_cache = {}


def _build():
    import concourse.bass as bass
    import concourse.mybir as mybir
    import concourse.bacc as bacc
    import concourse.tile as tile

    f32 = mybir.dt.float32
    bf16 = mybir.dt.bfloat16
    fp8 = mybir.dt.float8e4
    AF = mybir.ActivationFunctionType
    ALU = mybir.AluOpType
    PSUM = bass.MemorySpace.PSUM

    nc = bacc.Bacc(None, target_bir_lowering=False)

    # ---------------- DRAM parameters (host pre-swizzled; contiguous) ------
    xq_d = nc.dram_tensor("xq", [128, 16], bf16, kind="ExternalInput")
    identb_d = nc.dram_tensor("identb", [128, 128], bf16, kind="ExternalInput")
    identf_d = nc.dram_tensor("identf", [16, 16], f32, kind="ExternalInput")
    xo_d = nc.dram_tensor("xo", [1, SH], f32, kind="ExternalInput")

    wqa_d = nc.dram_tensor("wqa", [128, 2, 4096], fp8, kind="ExternalInput")
    wqb_d = nc.dram_tensor("wqb", [128, 2, 4096], fp8, kind="ExternalInput")
    wep_d = nc.dram_tensor("wep", [128, 3, 4096], fp8, kind="ExternalInput")

    kab_d = nc.dram_tensor("kab", [8, 128, 2, S_LOC], bf16, kind="ExternalInput")
    cksk_d = nc.dram_tensor("cksk", [8, 128, 2, S_LOC], bf16, kind="ExternalInput")
    st_d = nc.dram_tensor("st", [8, 128, VO], bf16, kind="ExternalInput")
    csss_d = nc.dram_tensor("csss", [8, 128, 2, HALF], bf16, kind="ExternalInput")

    out_d = nc.dram_tensor("out", [1, SH], f32, kind="ExternalOutput")

    RG = [list(range(NC))]
    SCALE_EXP = float(1.0 / np.sqrt(DQ) / (WSCALE ** 4))

    with tile.TileContext(nc) as tc:
        with (
            tc.tile_pool(name="wts", bufs=1) as wts,
            tc.tile_pool(name="kbuf", bufs=1) as kbuf,
            tc.tile_pool(name="sbuf_s", bufs=1) as sbuf_s,
            tc.tile_pool(name="tmps", bufs=1) as tmps,
            tc.tile_pool(name="small", bufs=1) as small,
            tc.tile_pool(name="psA", bufs=4, space=PSUM) as psA,
            tc.tile_pool(name="psB", bufs=4, space=PSUM) as psB,
            tc.tile_pool(name="dram", bufs=1, space="DRAM") as dram,
        ):
            # ---------------- collective bounce buffers (DRAM) ----------------
            bqh_in = dram.tile([128, NUM_HEADS], bf16)
            bqh_out = dram.tile([128, NUM_HEADS], bf16)
            bqt_in = [dram.tile([128, 8 * NUM_HEADS], bf16, name=f"bqt_in{g}")
                      for g in range(2)]
            bqt_out = [dram.tile([128, 8 * NUM_HEADS], bf16, name=f"bqt_out{g}")
                       for g in range(2)]
            bu_in = dram.tile([128, 16 * NUM_HEADS + 1], bf16)
            bu_out = dram.tile([128, 16 * NUM_HEADS + 1], bf16)
            bat_in = dram.tile([NUM_HEADS, 128], bf16)
            bat_out = dram.tile([128, 128], bf16)

            # ---------------- small persistent SBUF tiles ----------------
            x_sb = small.tile([128, 16], bf16, tag="x")
            identb = small.tile([128, 128], bf16, tag="idb")
            identf = small.tile([16, 16], f32, tag="idf")
            xo_sb = small.tile([1, SH], f32, tag="xo")
            qT_sb = small.tile([128, 2], bf16, tag="qT")
            qhTp_sb = small.tile([128, NUM_HEADS], bf16, tag="qhTp")
            qhT_sb = small.tile([128, NUM_HEADS], bf16, tag="qhT")
            tmpT_sb = small.tile([128, 2, NUM_HEADS], bf16, tag="tmpT")
            qtp_sb = small.tile([128, 16 * NUM_HEADS], bf16, tag="qtp")
            qtilT_sb = small.tile([128, 16, NUM_HEADS], bf16, tag="qtilT")
            w_sb = small.tile([NUM_HEADS, S_LOC], bf16, tag="w")
            l0_sb = small.tile([NUM_HEADS, 1], f32, tag="l0")
            l1_sb = small.tile([NUM_HEADS, 1], f32, tag="l1")
            lp_sb = small.tile([NUM_HEADS, 1], f32, tag="lp")
            wT_sb = small.tile([128, 8, NUM_HEADS], bf16, tag="wT")
            u_sb = small.tile([NUM_HEADS, VO], f32, tag="u")
            ub_sb = small.tile([128, 16 * NUM_HEADS + 1], bf16, tag="ub")
            uT_bf = small.tile([128, 16, NUM_HEADS], bf16, tag="uTb")
            l_sb = small.tile([NUM_HEADS, 1], bf16, tag="l")
            l16_sb = small.tile([NUM_HEADS, 1], f32, tag="l16")
            rl_sb = small.tile([NUM_HEADS, 1], f32, tag="rl")
            z_sb = small.tile([NUM_HEADS, SH], bf16, tag="z")
            zT_sb = small.tile([128, 2, NUM_HEADS], bf16, tag="zT")
            atT_sb = small.tile([128, NUM_HEADS], bf16, tag="atT")
            atr_sb = small.tile([NUM_HEADS, 128], bf16, tag="atr")
            aga_sb = small.tile([128, 128], bf16, tag="aga")
            agat_sb = small.tile([128, 128], bf16, tag="agat")
            atT_bf = small.tile([128, NUM_HEADS], bf16, tag="atTb")
            out_sb = small.tile([1, SH], f32, tag="out")

            # ================= DMA issue order (sync HWDGE ring) ===========
            # x + q-projection weights first: they gate the AR#1 trigger,
            # whose latest arrival across cores pins the rendezvous.
            wqa_sb = wts.tile([128, 2, 4096], fp8, tag="wqa")
            wqb_sb = wts.tile([128, 2, 4096], fp8, tag="wqb")
            nc.sync.dma_start(x_sb[:], xq_d[:, :])
            nc.sync.dma_start(wqa_sb[:], wqa_d[:, :, :])
            nc.sync.dma_start(identb[:], identb_d[:, :])
            nc.sync.dma_start(identf[:], identf_d[:, :])
            nc.sync.dma_start(xo_sb[:], xo_d[:, :])

            kab_t, cksk_t = [], []
            for ci in range(8):
                kab = kbuf.tile([128, 2, S_LOC], bf16, tag="kab", bufs=8, name=f"kab{ci}")
                cksk = kbuf.tile([128, 2, S_LOC], bf16, tag="cksk", bufs=4,
                                 name=f"cksk{ci}")
                nc.sync.dma_start(kab[:], kab_d[ci, :, :, :])
                nc.sync.dma_start(cksk[:], cksk_d[ci, :, :, :])
                kab_t.append(kab); cksk_t.append(cksk)

            # wqb (wkmC/wk) is needed only after AR#1 (~100us in) — load it
            # after the keys stream.
            nc.sync.dma_start(wqb_sb[:], wqb_d[:, :, :])

            st_t, csss_t = [], []
            for sb in range(8):
                stt = sbuf_s.tile([128, VO], bf16, tag="st", bufs=8, name=f"st{sb}")
                csss = sbuf_s.tile([128, 2, HALF], bf16, tag="csss", bufs=4,
                                   name=f"csss{sb}")
                nc.sync.dma_start(stt[:], st_d[sb, :, :])
                nc.sync.dma_start(csss[:], csss_d[sb, :, :, :])
                st_t.append(stt); csss_t.append(csss)

            wep_sb = wts.tile([128, 3, 4096], fp8, tag="wep")
            nc.sync.dma_start(wep_sb[:], wep_d[:, :, :])

            # weight slice helpers (flat fp8 packs)
            wqT_s = lambda kc, lo, hi: wqa_sb[:, 0, kc * 256 + lo : kc * 256 + hi]
            wqm_s = lambda n2, lo, hi: wqa_sb[:, 1, n2 * 2048 + lo : n2 * 2048 + hi]
            wkm_s = lambda h, lo, hi: wqb_sb[:, 0, h * 256 + lo : h * 256 + hi]
            wk_s = lambda jc, lo, hi: wqb_sb[:, 1, jc * 2048 + lo : jc * 2048 + hi]
            wvT_s = lambda ic, lo, hi: wep_sb[:, 0, ic * 256 + lo : ic * 256 + hi]
            wvm_s = lambda jc, lo, hi: wep_sb[:, 1, jc * 2048 + lo : jc * 2048 + hi]
            woT_s = lambda h, lo, hi: wep_sb[:, 2, h * 256 + lo : h * 256 + hi]

            # ================= q path =================
            for nc2 in range(2):
                qt_ps2 = psB.tile([128, 1], f32, tag="pB", name=f"qt_ps2_{nc2}")
                for kc in range(16):
                    nc.tensor.matmul(qt_ps2[:], wqT_s(kc, nc2 * 128, (nc2 + 1) * 128),
                                     x_sb[:, kc : kc + 1], start=(kc == 0), stop=(kc == 15))
                nc.scalar.activation(qT_sb[:, nc2 : nc2 + 1], qt_ps2[:], AF.Copy)

            qhT_ps = psB.tile([128, NUM_HEADS], f32, tag="pB")
            for h in range(NUM_HEADS):
                for nc2 in range(2):
                    nc.tensor.matmul(qhT_ps[:, h : h + 1],
                                     wqm_s(nc2, h * 128, (h + 1) * 128),
                                     qT_sb[:, nc2 : nc2 + 1],
                                     start=(nc2 == 0), stop=(nc2 == 1))
            nc.scalar.activation(qhTp_sb[:], qhT_ps[:], AF.Copy)
            nc.gpsimd.dma_start(bqh_in[:], qhTp_sb[:])
            nc.gpsimd.collective_compute(
                "AllReduce", ALU.add, ins=[bqh_in[:].opt()], outs=[bqh_out[:].opt()],
                replica_groups=RG)
            nc.gpsimd.dma_start(qhT_sb[:], bqh_out[:, :])

            tmpT_ps = [psB.tile([128, NUM_HEADS], f32, tag="pB", name=f"tmpT_ps{j}")
                       for j in range(2)]
            for h in range(NUM_HEADS):
                for jc in range(2):
                    nc.tensor.matmul(tmpT_ps[jc][:, h : h + 1],
                                     wkm_s(h, jc * 128, (jc + 1) * 128),
                                     qhT_sb[:, h : h + 1], start=True, stop=True)
            for jc in range(2):
                nc.scalar.activation(tmpT_sb[:, jc, :], tmpT_ps[jc][:], AF.Copy)

            # qtil partials: two halves, each with its own AllReduce so the
            # second mesh pipelines behind the first while logits half 1 runs
            qt_big = [psB.tile([128, 128], f32, tag="pB", name=f"qt_big{g}")
                      for g in range(2)]
            for ic in range(16):
                g, col = ic // 8, (ic % 8) * 16
                for jc in range(2):
                    nc.tensor.matmul(qt_big[g][:, col : col + 16],
                                     wk_s(jc, ic * 128, (ic + 1) * 128),
                                     tmpT_sb[:, jc, :], start=(jc == 0), stop=(jc == 1))
            for g in range(2):
                nc.scalar.activation(qtp_sb[:, g * 128 : (g + 1) * 128], qt_big[g][:],
                                     AF.Copy)
                nc.gpsimd.dma_start(bqt_in[g][:, :], qtp_sb[:, g * 128 : (g + 1) * 128])
                nc.gpsimd.collective_compute(
                    "AllReduce", ALU.add, ins=[bqt_in[g][:].opt()],
                    outs=[bqt_out[g][:].opt()], replica_groups=RG)
                nc.gpsimd.dma_start(
                    qtilT_sb[:, g * 8 : (g + 1) * 8, :],
                    bqt_out[g][:, :].rearrange("p (ic h) -> p ic h", ic=8))

            # ================= keys rope (classic, on DVE, pre-rendezvous) ==
            for ci in range(8):
                a = kab_t[ci][:, 0, :]
                b = kab_t[ci][:, 1, :]
                c = cksk_t[ci][:, 0, :]
                s = cksk_t[ci][:, 1, :]
                t1 = tmps.tile([128, S_LOC], bf16, tag="rt", bufs=8)
                t2 = tmps.tile([128, S_LOC], bf16, tag="rt", bufs=8)
                t3 = tmps.tile([128, S_LOC], bf16, tag="rt", bufs=8)
                t4 = tmps.tile([128, S_LOC], bf16, tag="rt", bufs=8)
                nc.vector.tensor_mul(t1[:], a, c)
                nc.vector.tensor_mul(t2[:], b, s)
                nc.vector.tensor_mul(t3[:], b, c)
                nc.vector.tensor_mul(t4[:], a, s)
                nc.vector.tensor_sub(a, t1[:], t2[:])
                nc.vector.tensor_add(b, t3[:], t4[:])

            # logits: 32 matmuls into two PSUM chunks
            lg_ps = [psA.tile([NUM_HEADS, 512], f32, tag="pA", name=f"lg{sc}")
                     for sc in range(2)]
            for hf in range(2):      # half 1 runs while AR#2b is still in flight
                for ci in range(8):
                    lhs = qtilT_sb[:, 8 * hf + ci, :]
                    for sc in range(2):
                        nc.tensor.matmul(
                            lg_ps[sc][:], lhs,
                            kab_t[ci][:, hf, sc * 512 : (sc + 1) * 512],
                            start=(hf == 0 and ci == 0), stop=(hf == 1 and ci == 7))

            # ================= states rope (classic, pre-rendezvous) ========
            for sb in range(8):
                stt = st_t[sb]
                c = csss_t[sb][:, 0, :]
                s = csss_t[sb][:, 1, :]
                t1 = tmps.tile([128, HALF], bf16, tag="rt", bufs=8)
                t2 = tmps.tile([128, HALF], bf16, tag="rt", bufs=8)
                t3 = tmps.tile([128, HALF], bf16, tag="rt", bufs=8)
                t4 = tmps.tile([128, HALF], bf16, tag="rt", bufs=8)
                nc.vector.tensor_mul(t1[:], stt[:, 0:HALF], c)
                nc.vector.tensor_mul(t2[:], stt[:, HALF:VO], s)
                nc.vector.tensor_mul(t3[:], stt[:, HALF:VO], c)
                nc.vector.tensor_mul(t4[:], stt[:, 0:HALF], s)
                nc.vector.tensor_sub(stt[:, 0:HALF], t1[:], t2[:])
                nc.vector.tensor_add(stt[:, HALF:VO], t3[:], t4[:])

            # exp + l, interleaved with wT transposes (PE) and bundled copies
            wt_ps = [psB.tile([128, 4 * NUM_HEADS], bf16, tag="pB", name=f"wt_ps{g}")
                     for g in range(2)]
            for sc in range(2):
                nc.scalar.activation(w_sb[:, sc * 512 : (sc + 1) * 512], lg_ps[sc][:],
                                     AF.Exp, scale=SCALE_EXP,
                                     accum_out=(l0_sb[:] if sc == 0 else l1_sb[:]))
                for k in range(4):
                    sb = sc * 4 + k
                    nc.tensor.transpose(wt_ps[sc][:, k * 16 : (k + 1) * 16],
                                        w_sb[:, sb * 128 : (sb + 1) * 128],
                                        identb[0:16, 0:16])
                nc.scalar.activation(
                    wT_sb[:, 4 * sc : 4 * (sc + 1), :].rearrange("p a h -> p (a h)"),
                    wt_ps[sc][:], AF.Copy)
            nc.vector.tensor_add(lp_sb[:], l0_sb[:], l1_sb[:])

            # u: 32 matmuls into four PSUM chunks
            u_ps = [psA.tile([NUM_HEADS, 512], f32, tag="pA", name=f"u_ps{i}")
                    for i in range(4)]
            for sb in range(8):
                for nch in range(4):
                    nc.tensor.matmul(u_ps[nch][:], wT_sb[:, sb, :],
                                     st_t[sb][:, nch * 512 : (nch + 1) * 512],
                                     start=(sb == 0), stop=(sb == 7))
            for nch in range(4):
                if nch % 2 == 0:
                    nc.scalar.activation(u_sb[:, nch * 512 : (nch + 1) * 512],
                                         u_ps[nch][:], AF.Copy)
                else:
                    nc.vector.tensor_copy(u_sb[:, nch * 512 : (nch + 1) * 512],
                                          u_ps[nch][:])

            # uT via PE transposes, bundled into 4 wide copies
            ut_ps = [psB.tile([128, 4 * NUM_HEADS], f32, tag="pB", name=f"ut_ps{g}")
                     for g in range(4)]
            for g in range(4):
                for k in range(4):
                    ic = g * 4 + k
                    nc.tensor.transpose(ut_ps[g][:, k * 16 : (k + 1) * 16],
                                        u_sb[:, ic * 128 : (ic + 1) * 128],
                                        identf[:, :])
                nc.scalar.activation(ub_sb[:, g * 64 : (g + 1) * 64], ut_ps[g][:],
                                     AF.Copy)
            nc.scalar.activation(ub_sb[0:NUM_HEADS, 256:257], lp_sb[:], AF.Copy)
            nc.gpsimd.dma_start(bu_in[:], ub_sb[:])
            nc.gpsimd.collective_compute(
                "AllReduce", ALU.add, ins=[bu_in[:].opt()], outs=[bu_out[:].opt()],
                replica_groups=RG)
            nc.gpsimd.dma_start(
                uT_bf[:], bu_out[:, 0:256].rearrange("p (ic h) -> p ic h", ic=16))
            nc.gpsimd.dma_start(l_sb[:], bu_out[0:NUM_HEADS, 256:257])
            nc.vector.tensor_scalar_mul(l16_sb[:], l_sb[:], WSCALE)
            nc.vector.reciprocal(rl_sb[:], l16_sb[:])

            # ================= epilogue =================
            z_ps = psB.tile([NUM_HEADS, SH], f32, tag="pB")
            for ic in range(16):
                nc.tensor.matmul(z_ps[:], uT_bf[:, ic, :], wvT_s(ic, 0, 256),
                                 start=(ic == 0), stop=(ic == 15))
            nc.scalar.activation(z_sb[:], z_ps[:], AF.Copy, scale=rl_sb[:])

            zt_ps = psB.tile([128, 2 * NUM_HEADS], bf16, tag="pB")
            for jc in range(2):
                nc.tensor.transpose(zt_ps[:, jc * 16 : (jc + 1) * 16],
                                    z_sb[:, jc * 128 : (jc + 1) * 128],
                                    identb[0:16, 0:16])
            nc.scalar.activation(zT_sb[:].rearrange("p a h -> p (a h)"), zt_ps[:],
                                 AF.Copy)

            at_ps = psB.tile([128, NUM_HEADS], f32, tag="pB")
            for h in range(NUM_HEADS):
                for jc in range(2):
                    nc.tensor.matmul(at_ps[:, h : h + 1],
                                     wvm_s(jc, h * 128, (h + 1) * 128),
                                     zT_sb[:, jc, h : h + 1],
                                     start=(jc == 0), stop=(jc == 1))
            # 1/256 descales wvm's x16 and pre-compensates woT's x16
            nc.scalar.activation(atT_sb[:], at_ps[:], AF.Copy,
                                 scale=float(1.0 / (WSCALE * WSCALE)))
            # transpose to [16, 128] for the partition-axis AllGather
            atr_ps = psB.tile([NUM_HEADS, 128], bf16, tag="pB")
            nc.tensor.transpose(atr_ps[:], atT_sb[:], identb[:, :])
            nc.scalar.activation(atr_sb[:], atr_ps[:], AF.Copy)
            nc.gpsimd.dma_start(bat_in[:], atr_sb[:])
            nc.gpsimd.collective_compute(
                "AllGather", ALU.bypass, ins=[bat_in[:].opt()], outs=[bat_out[:].opt()],
                replica_groups=RG)
            nc.gpsimd.dma_start(aga_sb[:], bat_out[:, :])
            # transpose the gathered [16r+h, d] blocks to [d, 16r+h], then the
            # 8 rank blocks become free-dim slices we can sum on DVE
            agat_ps = psB.tile([128, 128], bf16, tag="pB")
            nc.tensor.transpose(agat_ps[:], aga_sb[:], identb[:, :])
            nc.scalar.activation(agat_sb[:], agat_ps[:], AF.Copy)
            nc.vector.tensor_add(atT_bf[:], agat_sb[:, 0:16], agat_sb[:, 16:32])
            for r in range(2, 8):
                nc.vector.tensor_add(atT_bf[:], atT_bf[:],
                                     agat_sb[:, 16 * r : 16 * (r + 1)])

            o_ps = psB.tile([1, SH], f32, tag="pB")
            for h in range(NUM_HEADS):
                nc.tensor.matmul(o_ps[:], atT_bf[:, h : h + 1], woT_s(h, 0, 256),
                                 start=(h == 0), stop=(h == NUM_HEADS - 1))
            nc.vector.tensor_add(out_sb[:], o_ps[:], xo_sb[:])
            nc.sync.dma_start(out_d[:, :], out_sb[:])

    nc.compile()
    return nc


def _tables():
    half = HALF
    freqs = 1.0 / (ROPE_THETA ** (np.arange(half, dtype=np.float32) * 2.0 / VO))
    ang = np.outer(np.arange(S, dtype=np.float32), freqs).astype(np.float32)  # (S, half)
    return np.cos(ang), np.sin(ang)


def _w8flat(a, tiles):
    # [tiles*128, X] fp8-prescaled -> [128, tiles*X] (partition-contiguous)
    X = a.shape[1]
    sw = np.ascontiguousarray(
        (np.asarray(a, np.float32) * WSCALE).reshape(tiles, 128, X).transpose(1, 0, 2))
    return sw.reshape(128, tiles * X).astype(FP8)


def kernel(x, keys, states, Wq, Wk, Wv, Wq_mha, Wk_mha, Wv_mha, Wo):
    from concourse import bass_utils

    if "nc" not in _cache:
        _cache["nc"] = _build()
    nc = _cache["nc"]

    x = np.asarray(x, np.float32)
    keys = np.asarray(keys, np.float32)
    states = np.asarray(states, np.float32)
    cos_t, sin_t = _tables()

    xq2d = np.ascontiguousarray(x.reshape(16, 128).T).astype(BF16)
    ident128b = np.eye(128, dtype=np.float32).astype(BF16)
    ident16f = np.eye(16, dtype=np.float32)

    in_maps = []
    for c in range(NC):
        rs = slice(c * SH, (c + 1) * SH)
        ssl = slice(c * S_LOC, (c + 1) * S_LOC)
        cosc = cos_t[ssl]            # (1024, 1024) [s_loc, j]
        sinc = sin_t[ssl]

        wqa = np.stack([
            _w8flat(Wq[rs].T, 16),
            _w8flat(Wq_mha[:, rs].T, 2),
        ], axis=1)                   # [128, 2, 4096]
        wqb = np.stack([
            _w8flat(Wk_mha[:, rs], 16),
            _w8flat(Wk[rs], 2),
        ], axis=1)                   # [128, 2, 4096]
        wep = np.stack([
            _w8flat(Wv[rs].T, 16),
            _w8flat(Wv_mha[:, rs].T, 2),
            _w8flat(Wo[rs].T, 16),
        ], axis=1)                   # [128, 3, 4096]

        kT = keys[ssl].T.astype(BF16).reshape(16, 128, S_LOC)
        kab = np.stack([kT[0:8], kT[8:16]], axis=2)          # [8, 128, 2, 1024]
        cT = cosc.T.astype(BF16).reshape(8, 128, S_LOC)
        sT = sinc.T.astype(BF16).reshape(8, 128, S_LOC)
        cksk = np.stack([cT, sT], axis=2)                    # [8, 128, 2, 1024]
        cN = cosc.astype(BF16).reshape(8, 128, HALF)
        sN = sinc.astype(BF16).reshape(8, 128, HALF)
        csss = np.stack([cN, sN], axis=2)                    # [8, 128, 2, 1024]

        m = {
            "xq": xq2d,
            "identb": ident128b,
            "identf": ident16f,
            "xo": np.ascontiguousarray(x[rs]).reshape(1, SH),
            "wqa": np.ascontiguousarray(wqa),
            "wqb": np.ascontiguousarray(wqb),
            "wep": np.ascontiguousarray(wep),
            "kab": np.ascontiguousarray(kab),
            "cksk": np.ascontiguousarray(cksk),
            "st": np.ascontiguousarray(states[ssl]).astype(BF16).reshape(8, 128, VO),
            "csss": np.ascontiguousarray(csss),
        }
        in_maps.append(m)

    global _last_in_maps, _last_res
    _last_in_maps = in_maps
    res = bass_utils.run_bass_kernel_spmd(nc, in_maps, core_ids=list(range(NC)))
    _last_res = res
    out = np.concatenate([np.asarray(res.results[c]["out"]).reshape(-1) for c in range(NC)])
    return out[None, :].astype(np.float32)
